# revision 20
# baseline (speedup 1.0000x reference)
"""Trainium2 Bass kernel for nn_MultiHeadAttention (B=2, S=2048, d_model=1024, H=16).

Sharding (8 cores): data-parallel over B (2) x tensor-parallel over head groups
(4 groups of 4 heads).  Each core computes its head-group's Q/K/V projections
(column-sharded weights), attention for its 4 heads, and a row-parallel
out_proj partial product.  The host sums the 4 partials per batch (the
"all-reduce") and adds the output bias.

v3 design notes (cost-model driven):
  - P@V uses SWAPPED operands: P (exp scores, [k,q]) stationary, V ([k,e])
    moving, so the moving free dim is 64 instead of 512; attention output
    lands in [q, e].  Softmax denominators ride along as N=1 matmuls
    (rhs = ones) reusing the loaded P stationary tile.
  - Normalization = per-partition scalar multiply on DVE; PE transposes
    bring [q, e] back to [e, q] for the row-parallel out_proj.
  - PSUM is a single 8-bank working set shared by EVERYTHING (no stacked
    stage pools, which would serialize projections before attention):
    sc 2x2 banks, out2 2, sums 1, scratch 1.  Projections beyond the
    first k/q n-chunk are drip-fed through the scratch bank inside the
    attention loop (deadline-ordered backlog), so the ACT exp stream --
    the critical resource -- starts ~15us in instead of ~65us.
  - x is loaded in [128, d, 512] n-chunks (one DMA each) so the first
    chunk of K and Q arrives after ~9us of serial DMA instead of ~30us.
  - bk is dropped exactly (softmax shift invariance); bv is folded into
    the host-side output bias (attention weights sum to 1); bq is applied
    on-device during the qh PSUM->SBUF copy.
  - PSUM start_tensor_calc zeroing is bank-granular: only the first
    matmul touching a bank in an accumulation group sets start=True.
"""

import sys
import numpy as np

for _p in ("/opt/trn_rl_repo", "/root/.axon_site/_ro/trn_rl_repo"):
    if _p not in sys.path:
        sys.path.append(_p)

D_MODEL = 1024
NUM_HEADS = 16
DK = 64
B = 2
S = 2048
N_CORES = 8
HPC = 4               # heads per core
E = HPC * DK          # 256 features per core
NQ = 512              # q-chunk size
N_QC = S // NQ        # 4 q chunks
N_KT = S // 128       # 16 k tiles
N_DT = D_MODEL // 128  # 8 contraction tiles for projections

_PROGRAM = None
_RUN_KWARGS = {}      # test harness may set {"trace": True}
_LAST_RESULTS = None  # BassKernelResults of the last run

# Backlog draw schedule: how many deferred projection chunks to emit
# after each (qc, kt) iteration of the attention loop.
_DRAW = {0: [1, 1, 1, 1, 1, 1, 1, 1, 1, 2, 1, 2, 2, 2, 1, 1],
         1: [1, 1, 1, 1] + [0] * 12}


def _build_program():
    import concourse.bass as bass
    import concourse.mybir as mybir
    from concourse import bacc, tile
    from contextlib import ExitStack

    f32 = mybir.dt.float32
    fp16 = mybir.dt.float16
    i16 = mybir.dt.int16
    AF = mybir.ActivationFunctionType
    ALU = mybir.AluOpType
    # Schraudolph fast-exp constants (int16/fp16 bitcast):
    #   i16 = round(s * 0.125 * 1024/ln2 + (15*1024 - C));  C tuned for
    #   min RMS rel error (~1.8%); applied to ~22% of exp tiles on DVE.
    SCH_A = 0.125 * 1024.0 / np.log(2.0)
    SCH_B = 15.0 * 1024.0 - 60.0

    nc = bacc.Bacc("TRN2", target_bir_lowering=False, debug=False,
                   num_devices=N_CORES)

    fp8 = mybir.dt.float8e4
    DR = mybir.MatmulPerfMode.DoubleRow
    xdr = {}
    for nm in ("qTh", "qTl", "kTh", "kTl", "vTh", "vTl"):
        xdr[nm] = nc.dram_tensor(nm, [D_MODEL, S], fp8,
                                 kind="ExternalInput").ap()
    wdr = {}
    for nm in ("wqh", "wql", "wkh", "wkl", "wvh", "wvl"):
        wdr[nm] = nc.dram_tensor(nm, [D_MODEL, E], fp8,
                                 kind="ExternalInput").ap()
    wo = nc.dram_tensor("wo", [E, D_MODEL], fp16, kind="ExternalInput").ap()
    bq = nc.dram_tensor("bq", [E, 1], f32, kind="ExternalInput").ap()
    onesk = nc.dram_tensor("onesk", [128, 1], fp16, kind="ExternalInput").ap()
    zT = nc.dram_tensor("zT", [D_MODEL, S], fp16, kind="ExternalOutput").ap()

    with tile.TileContext(nc) as tc, ExitStack() as ctx:
        persist = ctx.enter_context(tc.tile_pool(name="persist", bufs=1))
        const = ctx.enter_context(tc.tile_pool(name="const", bufs=1))

        w_sb = {}
        for nm in ("wvh", "wvl", "wkh", "wkl", "wqh", "wql"):
            w_sb[nm] = persist.tile([128, N_DT, E], fp8, tag=nm, name=nm)
        wo_sb = persist.tile([128, 2, D_MODEL], fp16, tag="wo", name="wo")
        bq_sb = persist.tile([128, 2], f32, tag="bq", name="bq")

        from concourse.masks import make_identity
        ident = const.tile([128, 128], fp16, tag="ident", name="ident")
        make_identity(nc, ident)
        ones_k = const.tile([128, 1], fp16, tag="ones_k", name="ones_k")

        qh = [persist.tile([128, S], fp16, tag=f"qh{p}", name=f"qh{p}")
              for p in range(2)]
        kh = [persist.tile([128, S], fp16, tag=f"kh{p}", name=f"kh{p}")
              for p in range(2)]
        vh = persist.tile([128, N_KT, E], fp16, tag="vh", name="vh")
        ot = [persist.tile([128, S], fp16, tag=f"ot{p}", name=f"ot{p}")
              for p in range(2)]

        # ---- x chunk tiles + DMA schedule (priority order) --------------
        xpool = ctx.enter_context(tc.tile_pool(name="xpool", bufs=24))
        xt = {}
        for t in ("k", "q", "v"):
            for hl in "hl":
                xt[t + hl] = [xpool.tile([128, N_DT, NQ], fp8, tag="xt",
                                         name=f"x{t}{hl}{n}")
                              for n in range(4)]
        x3 = {k: v.rearrange("(t p) s -> p t s", p=128)
              for k, v in xdr.items()}

        def _xdma(eng, t, hl, n):
            eng.dma_start(xt[t + hl][n][:],
                          x3[t + "T" + hl][:, :, n * NQ:(n + 1) * NQ])

        def _wdma(eng, nm):
            eng.dma_start(w_sb[nm][:],
                          wdr[nm].rearrange("(t p) e -> p t e", p=128))

        # Everything on the SP queue: transfers serialize on the shared DMA
        # device regardless, and any DMA on the scalar queue would block the
        # ACT sequencer from issuing the (critical) exp stream.
        _wdma(nc.sync, "wkh")
        _wdma(nc.sync, "wkl")
        _xdma(nc.sync, "k", "h", 0)
        _xdma(nc.sync, "k", "l", 0)
        _wdma(nc.sync, "wvh")
        _wdma(nc.sync, "wvl")
        nc.sync.dma_start(ones_k[:], onesk)
        _xdma(nc.sync, "v", "h", 0)
        _xdma(nc.sync, "v", "l", 0)
        _wdma(nc.sync, "wqh")
        _wdma(nc.sync, "wql")
        _xdma(nc.sync, "q", "h", 0)
        _xdma(nc.sync, "q", "l", 0)
        nc.sync.dma_start(bq_sb[:], bq.rearrange("(m p) o -> p (m o)", p=128))
        for n in range(1, 4):
            _xdma(nc.sync, "v", "h", n)
            _xdma(nc.sync, "v", "l", n)
            _xdma(nc.sync, "k", "h", n)
            _xdma(nc.sync, "k", "l", n)
            _xdma(nc.sync, "q", "h", n)
            _xdma(nc.sync, "q", "l", n)
        nc.sync.dma_start(wo_sb[:], wo.rearrange("(t p) e -> p t e", p=128))

        # ---- PSUM pools: one shared 8-bank working set ------------------
        scp = ctx.enter_context(tc.tile_pool(name="scp", bufs=4, space="PSUM"))
        outp = ctx.enter_context(tc.tile_pool(name="outp", bufs=1, space="PSUM"))
        sump = ctx.enter_context(tc.tile_pool(name="sump", bufs=1, space="PSUM"))
        scr = ctx.enter_context(tc.tile_pool(name="scr", bufs=1, space="PSUM"))

        ptp = ctx.enter_context(tc.tile_pool(name="ptp", bufs=28))
        rp = ctx.enter_context(tc.tile_pool(name="rp", bufs=2))
        bcp = ctx.enter_context(tc.tile_pool(name="bcp", bufs=8))
        zsbp = ctx.enter_context(tc.tile_pool(name="zsbp", bufs=4))

        # ---- first K/Q n-chunk on the (still idle) score slots ----------
        # weights are host-scaled by 32 (fp8e4 subnormal avoidance); the
        # PSUM->SBUF copy applies the 1/32.  3 passes: wh@xh + wh@xl + wl@xh.
        def _dr_passes(t, wn):
            return ((w_sb[wn + "h"], xt[t + "h"]),
                    (w_sb[wn + "h"], xt[t + "l"]),
                    (w_sb[wn + "l"], xt[t + "h"]))

        def proj_big(t, wn, dst, n, bias):
            for m in range(2):
                ps = scp.tile([128, NQ], f32, tag="sc", name="projbig")
                passes = _dr_passes(t, wn)
                for pi, (wsb, xs) in enumerate(passes):
                    for dp in range(N_DT // 2):
                        nc.tensor.matmul(
                            ps[:], wsb[:, 2 * dp:2 * dp + 2,
                                       m * 128:(m + 1) * 128],
                            xs[n][:, 2 * dp:2 * dp + 2, :],
                            start=(pi == 0 and dp == 0),
                            stop=(pi == 2 and dp == N_DT // 2 - 1),
                            perf_mode=DR)
                if bias is None:
                    nc.vector.tensor_scalar_mul(
                        dst[m][:, n * NQ:(n + 1) * NQ], ps[:], 1.0 / 32)
                else:
                    nc.vector.tensor_scalar(
                        dst[m][:, n * NQ:(n + 1) * NQ], ps[:], 1.0 / 32,
                        bias[:, m:m + 1], ALU.mult, ALU.add)

        # V0-3 run during the kT/qT DMA wait and warm up the PE p-state
        # (they only need wv + the first vT chunk, which load first).
        # Dummy identity transposes (never read) fill the remaining DMA-wait
        # gaps so the p-state ramp reaches full speed before Kn0/Qn0.
        _V_PRE = 4
        wtp = scp.tile([128, 1024], fp16, tag="sc", name="wtp")

        def warm(cnt):
            for i in range(cnt):
                nc.tensor.matmul(
                    wtp[:, (i % 8) * 128:(i % 8 + 1) * 128], ident[:],
                    ident[:], is_transpose=True, start=True, stop=True,
                    skip_group_check=True)

        # ---- deferred projection backlog (drip-fed through scratch) -----
        def emit_v(st):
            vps = scr.tile([128, E], f32, tag="scr", name="vps")
            n, col = st // 4, (st % 4) * 128
            passes = ((xt["vh"], w_sb["wvh"]), (xt["vl"], w_sb["wvh"]),
                      (xt["vh"], w_sb["wvl"]))
            for pi, (xs, wsb) in enumerate(passes):
                for dp in range(N_DT // 2):
                    nc.tensor.matmul(
                        vps[:], xs[n][:, 2 * dp:2 * dp + 2, col:col + 128],
                        wsb[:, 2 * dp:2 * dp + 2, :],
                        start=(pi == 0 and dp == 0),
                        stop=(pi == 2 and dp == N_DT // 2 - 1),
                        perf_mode=DR)
            nc.vector.tensor_scalar_mul(vh[:, st, :], vps[:], 1.0 / 32)

        def emit_kq_chunk(t, wn, dst, n, m, bias):
            ps = scr.tile([128, NQ], f32, tag="scr", name="kqps")
            passes = _dr_passes(t, wn)
            for pi, (wsb, xs) in enumerate(passes):
                for dp in range(N_DT // 2):
                    nc.tensor.matmul(
                        ps[:], wsb[:, 2 * dp:2 * dp + 2,
                                   m * 128:(m + 1) * 128],
                        xs[n][:, 2 * dp:2 * dp + 2, :],
                        start=(pi == 0 and dp == 0),
                        stop=(pi == 2 and dp == N_DT // 2 - 1),
                        perf_mode=DR)
            if bias is None:
                nc.vector.tensor_scalar_mul(
                    dst[m][:, n * NQ:(n + 1) * NQ], ps[:], 1.0 / 32)
            else:
                nc.vector.tensor_scalar(
                    dst[m][:, n * NQ:(n + 1) * NQ], ps[:], 1.0 / 32,
                    bias[:, m:m + 1], ALU.mult, ALU.add)

        warm(20)
        proj_big("k", "wk", kh, 0, None)
        warm(8)
        for st in range(_V_PRE):
            emit_v(st)
        warm(8)
        proj_big("q", "wq", qh, 0, bq_sb)

        backlog = []
        _K = lambda n, m: (lambda: emit_kq_chunk("k", "wk", kh, n, m, None))
        _Q = lambda n, m: (lambda: emit_kq_chunk("q", "wq", qh, n, m, bq_sb))
        _V = lambda st: (lambda: emit_v(st))
        backlog += [_V(4), _K(1, 0), _V(5), _K(1, 1), _V(6), _K(2, 0),
                    _V(7), _K(2, 1), _V(8), _V(9), _K(3, 0), _V(10),
                    _V(11), _K(3, 1), _V(12), _Q(1, 0), _V(13), _Q(1, 1),
                    _V(14), _V(15), _Q(2, 0), _Q(2, 1), _Q(3, 0), _Q(3, 1)]
        backlog = backlog[::-1]  # pop() from the front

        # ---- attention + out_proj, per q-chunk --------------------------
        for qc in range(N_QC):
            q0, q1 = qc * NQ, (qc + 1) * NQ
            out2 = outp.tile([128, 4, E], f32, tag="out2", name="out2")
            sums = sump.tile([128, 16], f32, tag="sums", name="sums")

            def pv_sums(kt, pts):
                # only the FIRST matmul touching each PSUM bank of an
                # accumulation group may set start=True (bank-granular zero)
                for h in range(4):
                    for qt in range(4):
                        lhsT = pts[h][:, qt * 128:(qt + 1) * 128]
                        nc.tensor.matmul(
                            out2[:, qt, h * 64:(h + 1) * 64], lhsT,
                            vh[:, kt, h * 64:(h + 1) * 64],
                            start=(kt == 0 and h == 0 and qt % 2 == 0),
                            stop=(kt == N_KT - 1),
                            skip_group_check=True)
                        nc.tensor.matmul(
                            sums[:, qt * 4 + h:qt * 4 + h + 1], lhsT,
                            ones_k[:],
                            start=(kt == 0 and h == 0 and qt == 0),
                            stop=(kt == N_KT - 1),
                            skip_group_check=True)

            draw = _DRAW.get(qc, [0] * N_KT)
            prev_pts = None
            for kt in range(N_KT):
                k0 = kt * 128
                scs = []
                for h in range(4):
                    p, j = h // 2, h % 2
                    lo, hi = j * 64, (j + 1) * 64
                    sc = scp.tile([128, NQ], f32, tag="sc", name="sc")
                    nc.tensor.matmul(
                        sc[:], kh[p][lo:hi, k0:k0 + 128],
                        qh[p][lo:hi, q0:q1], start=True, stop=True)
                    scs.append(sc)
                if prev_pts is not None:
                    pv_sums(kt - 1, prev_pts)
                for _ in range(draw[kt]):
                    if backlog:
                        backlog.pop()()
                pts = []
                for h in range(4):
                    off = ((kt * 4 + h) % 16 == 9 if qc == 0
                           else (kt * 4 + h) % 8 in (1, 4, 6))
                    if off:
                        pti = ptp.tile([128, NQ], i16, tag="pt", name="pti")
                        nc.vector.tensor_scalar(
                            pti[:], scs[h][:], SCH_A, SCH_B,
                            ALU.mult, ALU.add)
                        pt = pti.bitcast(fp16)
                    else:
                        pt = ptp.tile([128, NQ], fp16, tag="pt", name="pt")
                        nc.scalar.activation(pt[:], scs[h][:], AF.Exp,
                                             scale=0.125)
                    pts.append(pt)
                prev_pts = pts
            pv_sums(N_KT - 1, prev_pts)

            # drain: normalize in [q, e], transpose to [e, q], out_proj.
            # All 16 normalize blocks go first (DVE/ACT alternating on the
            # last chunk), then all transposes, then all ot copies, so the
            # engines pipeline instead of chaining.  The out_proj of qc<3 is
            # deferred into the NEXT q-chunk's backlog (so it never blocks
            # the loop); qc3's runs on the freed score slots.
            last = qc == N_QC - 1
            rv = rp.tile([128, 16], f32, tag="rv", name="rv")
            nc.vector.reciprocal(rv[:], sums[:])
            o2n = []
            for qt in range(4):
                o2 = bcp.tile([128, E], fp16, tag="o2n", name="o2n")
                o2n.append(o2)
            for qt in range(4):
                for h in range(4):
                    c0 = qt * 4 + h
                    if last and h % 2 == 1:
                        nc.scalar.activation(
                            o2n[qt][:, h * 64:(h + 1) * 64],
                            out2[:, qt, h * 64:(h + 1) * 64],
                            AF.Copy, scale=rv[:, c0:c0 + 1])
                    else:
                        nc.vector.tensor_scalar_mul(
                            o2n[qt][:, h * 64:(h + 1) * 64],
                            out2[:, qt, h * 64:(h + 1) * 64],
                            rv[:, c0:c0 + 1])
            tp = scr.tile([128, 1024], fp16, tag="scr", name="tp")
            for qt in range(4):
                for et in range(2):
                    blk = qt * 2 + et
                    nc.tensor.matmul(
                        tp[:, blk * 128:(blk + 1) * 128],
                        o2n[qt][:, et * 128:(et + 1) * 128], ident[:],
                        is_transpose=True, start=True, stop=True,
                        skip_group_check=True)
            for qt in range(4):
                for et in range(2):
                    blk = qt * 2 + et
                    if last and blk % 2 == 1:
                        nc.scalar.activation(
                            ot[et][:, q0 + qt * 128:q0 + (qt + 1) * 128],
                            tp[:, blk * 128:(blk + 1) * 128],
                            AF.Copy, scale=1.0)
                    else:
                        nc.vector.tensor_copy(
                            ot[et][:, q0 + qt * 128:q0 + (qt + 1) * 128],
                            tp[:, blk * 128:(blk + 1) * 128])

            def out_proj_chunk(qc_, e, on_sc):
                q0_, q1_ = qc_ * NQ, (qc_ + 1) * NQ
                if on_sc:
                    zps = scp.tile([128, NQ], f32, tag="sc", name="zps")
                else:
                    zps = scr.tile([128, NQ], f32, tag="scr", name="zps")
                for c in range(2):
                    nc.tensor.matmul(
                        zps[:], wo_sb[:, c, e * 128:(e + 1) * 128],
                        ot[c][:, q0_:q1_], start=(c == 0), stop=(c == 1))
                zsb = zsbp.tile([128, NQ], fp16, tag="zsb", name="zsb")
                if on_sc and e % 2 == 1:
                    nc.scalar.activation(zsb[:], zps[:], AF.Copy, scale=1.0)
                else:
                    nc.vector.tensor_copy(zsb[:], zps[:])
                nc.sync.dma_start(zT[e * 128:(e + 1) * 128, q0_:q1_], zsb[:])

            if last:
                for e in range(N_DT):
                    out_proj_chunk(qc, e, True)
            else:
                for e in range(N_DT):
                    out_proj_chunk(qc, e, False)

    nc.compile()
    return nc


def _get_program():
    global _PROGRAM
    if _PROGRAM is None:
        _PROGRAM = _build_program()
    return _PROGRAM


ONESK_NP = None


def _init_consts():
    global ONESK_NP
    if ONESK_NP is None:
        ONESK_NP = np.ones((128, 1), np.float16)


def _hilo(a, f8):
    hi = a.astype(f8)
    lo = (a - hi.astype(np.float32)).astype(f8)
    return np.ascontiguousarray(hi), np.ascontiguousarray(lo)


def _make_in_maps(q, k, v, Wq, bq, Wk, Wv, Wo):
    _init_consts()
    import ml_dtypes
    f8 = ml_dtypes.float8_e4m3
    f32 = np.float32
    xT = {}
    for b in range(B):
        for nm, x in (("q", q), ("k", k), ("v", v)):
            h, lo = _hilo(np.ascontiguousarray(x[b].T), f8)
            xT[(nm, b)] = (h, lo)
    wslices = {}
    for g in range(4):
        sl = slice(g * E, (g + 1) * E)
        for nm, W in (("wq", Wq), ("wk", Wk), ("wv", Wv)):
            h, lo = _hilo(np.ascontiguousarray(W[sl, :].T) * 32.0, f8)
            wslices[(nm, g)] = (h, lo)
        wslices[("wo", g)] = np.ascontiguousarray(Wo[:, sl].T, dtype=np.float16)
        wslices[("bq", g)] = np.ascontiguousarray(
            bq[sl].reshape(E, 1), dtype=f32)
    in_maps = []
    for c in range(N_CORES):
        b, g = c // 4, c % 4
        in_maps.append({
            "onesk": ONESK_NP,
            "qTh": xT[("q", b)][0], "qTl": xT[("q", b)][1],
            "kTh": xT[("k", b)][0], "kTl": xT[("k", b)][1],
            "vTh": xT[("v", b)][0], "vTl": xT[("v", b)][1],
            "wqh": wslices[("wq", g)][0], "wql": wslices[("wq", g)][1],
            "wkh": wslices[("wk", g)][0], "wkl": wslices[("wk", g)][1],
            "wvh": wslices[("wv", g)][0], "wvl": wslices[("wv", g)][1],
            "wo": wslices[("wo", g)], "bq": wslices[("bq", g)],
        })
    return in_maps


def _numpy_fallback(q, k, v, mask, Wq, bq, Wk, bk, Wv, bv, Wo, bo):
    # Only used if mask is not all-True (never the case for this problem).
    def proj(x, W, b_):
        y = x @ W.T + b_
        return y.reshape(B, S, NUM_HEADS, DK).transpose(0, 2, 1, 3)
    qh, kh, vh = proj(q, Wq, bq), proj(k, Wk, bk), proj(v, Wv, bv)
    sc = np.einsum("bhqd,bhkd->bhqk", qh, kh) / np.sqrt(DK)
    sc = np.where(mask, sc, np.float32(-1e9))
    sc = sc - sc.max(-1, keepdims=True)
    p = np.exp(sc)
    p /= p.sum(-1, keepdims=True)
    o = np.einsum("bhqk,bhkd->bhqd", p, vh)
    o = o.transpose(0, 2, 1, 3).reshape(B, S, D_MODEL)
    return (o @ Wo.T + bo).astype(np.float32)


def kernel(q, k, v, mask, Wq, bq, Wk, bk, Wv, bv, Wo, bo):
    q = np.asarray(q, dtype=np.float32)
    k = np.asarray(k, dtype=np.float32)
    v = np.asarray(v, dtype=np.float32)
    Wq, Wk, Wv, Wo = (np.asarray(w, dtype=np.float32) for w in (Wq, Wk, Wv, Wo))
    bq, bk, bv, bo = (np.asarray(x, dtype=np.float32) for x in (bq, bk, bv, bo))
    if not np.all(np.asarray(mask)):
        return _numpy_fallback(q, k, v, np.asarray(mask), Wq, bq, Wk, bk,
                               Wv, bv, Wo, bo)

    from concourse.bass_utils import run_bass_kernel_spmd
    nc = _get_program()
    in_maps = _make_in_maps(q, k, v, Wq, bq, Wk, Wv, Wo)
    res = run_bass_kernel_spmd(nc, in_maps, core_ids=list(range(N_CORES)),
                               **_RUN_KWARGS)
    global _LAST_RESULTS
    _LAST_RESULTS = res
    # bk is dropped on-device (exact: softmax shift invariance); bv is
    # folded into the output bias (attention weights sum to 1).
    bo_eff = bo + Wo @ bv
    out = np.empty((B, S, D_MODEL), dtype=np.float32)
    for b in range(B):
        acc = res.results[4 * b]["zT"].astype(np.float32)
        for g in range(1, 4):
            acc = acc + res.results[4 * b + g]["zT"].astype(np.float32)
        out[b] = acc.T + bo_eff
    return out


# revision 27
# speedup vs baseline: 1.0046x; 1.0046x over previous
"""Trainium2 Bass kernel for nn_MultiHeadAttention (B=2, S=2048, d_model=1024, H=16).

Sharding (8 cores): data-parallel over B (2) x tensor-parallel over head groups
(4 groups of 4 heads).  Each core computes its head-group's Q/K/V projections
(column-sharded weights), attention for its 4 heads, and a row-parallel
out_proj partial product.  The host sums the 4 partials per batch (the
"all-reduce") and adds the output bias.

v3 design notes (cost-model driven):
  - P@V uses SWAPPED operands: P (exp scores, [k,q]) stationary, V ([k,e])
    moving, so the moving free dim is 64 instead of 512; attention output
    lands in [q, e].  Softmax denominators ride along as N=1 matmuls
    (rhs = ones) reusing the loaded P stationary tile.
  - Normalization = per-partition scalar multiply on DVE; PE transposes
    bring [q, e] back to [e, q] for the row-parallel out_proj.
  - PSUM is a single 8-bank working set shared by EVERYTHING (no stacked
    stage pools, which would serialize projections before attention):
    sc 2x2 banks, out2 2, sums 1, scratch 1.  Projections beyond the
    first k/q n-chunk are drip-fed through the scratch bank inside the
    attention loop (deadline-ordered backlog), so the ACT exp stream --
    the critical resource -- starts ~15us in instead of ~65us.
  - x is loaded in [128, d, 512] n-chunks (one DMA each) so the first
    chunk of K and Q arrives after ~9us of serial DMA instead of ~30us.
  - bk is dropped exactly (softmax shift invariance); bv is folded into
    the host-side output bias (attention weights sum to 1); bq is applied
    on-device during the qh PSUM->SBUF copy.
  - PSUM start_tensor_calc zeroing is bank-granular: only the first
    matmul touching a bank in an accumulation group sets start=True.
"""

import sys
import numpy as np

for _p in ("/opt/trn_rl_repo", "/root/.axon_site/_ro/trn_rl_repo"):
    if _p not in sys.path:
        sys.path.append(_p)

D_MODEL = 1024
NUM_HEADS = 16
DK = 64
B = 2
S = 2048
N_CORES = 8
HPC = 4               # heads per core
E = HPC * DK          # 256 features per core
NQ = 512              # q-chunk size
N_QC = S // NQ        # 4 q chunks
N_KT = S // 128       # 16 k tiles
N_DT = D_MODEL // 128  # 8 contraction tiles for projections

_PROGRAM = None
_RUN_KWARGS = {}      # test harness may set {"trace": True}
_LAST_RESULTS = None  # BassKernelResults of the last run

# Backlog draw schedule: how many deferred projection chunks to emit
# after each (qc, kt) iteration of the attention loop.
_DRAW = {0: [1, 1, 1, 1, 1, 1, 1, 1, 1, 1, 1, 1, 1, 1, 0, 0],
         1: [1, 1, 1, 1] + [0] * 12}


def _build_program():
    import concourse.bass as bass
    import concourse.mybir as mybir
    from concourse import bacc, tile
    from contextlib import ExitStack

    f32 = mybir.dt.float32
    fp16 = mybir.dt.float16
    i16 = mybir.dt.int16
    AF = mybir.ActivationFunctionType
    ALU = mybir.AluOpType
    # Schraudolph fast-exp constants (int16/fp16 bitcast):
    #   i16 = round(s * 0.125 * 1024/ln2 + (15*1024 - C));  C tuned for
    #   min RMS rel error (~1.8%); applied to ~22% of exp tiles on DVE.
    SCH_A = 0.125 * 1024.0 / np.log(2.0)
    SCH_B = 15.0 * 1024.0 - 60.0

    nc = bacc.Bacc("TRN2", target_bir_lowering=False, debug=False,
                   num_devices=N_CORES)

    fp8 = mybir.dt.float8e4
    DR = mybir.MatmulPerfMode.DoubleRow
    xdr = {}
    for nm in ("qTh", "qTl", "kTh", "kTl", "vTh", "vTl"):
        xdr[nm] = nc.dram_tensor(nm, [D_MODEL, S], fp8,
                                 kind="ExternalInput").ap()
    wdr = {}
    for nm in ("wqh", "wql", "wkh", "wkl", "wvh", "wvl"):
        wdr[nm] = nc.dram_tensor(nm, [D_MODEL, E], fp8,
                                 kind="ExternalInput").ap()
    wo = nc.dram_tensor("wo", [E, D_MODEL], fp16, kind="ExternalInput").ap()
    bq = nc.dram_tensor("bq", [E, 1], f32, kind="ExternalInput").ap()
    onesk = nc.dram_tensor("onesk", [128, 1], fp16, kind="ExternalInput").ap()
    zT = nc.dram_tensor("zT", [D_MODEL, S], fp16, kind="ExternalOutput").ap()

    with tile.TileContext(nc) as tc, ExitStack() as ctx:
        persist = ctx.enter_context(tc.tile_pool(name="persist", bufs=1))
        const = ctx.enter_context(tc.tile_pool(name="const", bufs=1))

        w_sb = {}
        for nm in ("wvh", "wvl", "wkh", "wkl", "wqh", "wql"):
            w_sb[nm] = persist.tile([128, N_DT, E], fp8, tag=nm, name=nm)
        wo_sb = persist.tile([128, 2, D_MODEL], fp16, tag="wo", name="wo")
        bq_sb = persist.tile([128, 2], f32, tag="bq", name="bq")

        from concourse.masks import make_identity
        ident = const.tile([128, 128], fp16, tag="ident", name="ident")
        make_identity(nc, ident)
        ones_k = const.tile([128, 1], fp16, tag="ones_k", name="ones_k")

        qh = [persist.tile([128, S], fp16, tag=f"qh{p}", name=f"qh{p}")
              for p in range(2)]
        kh = [persist.tile([128, S], fp16, tag=f"kh{p}", name=f"kh{p}")
              for p in range(2)]
        vh = persist.tile([128, N_KT, E], fp16, tag="vh", name="vh")
        ot = [persist.tile([128, S], fp16, tag=f"ot{p}", name=f"ot{p}")
              for p in range(2)]

        # ---- x chunk tiles + DMA schedule (priority order) --------------
        xpool = ctx.enter_context(tc.tile_pool(name="xpool", bufs=24))
        xt = {}
        for t in ("k", "q", "v"):
            for hl in "hl":
                xt[t + hl] = [xpool.tile([128, N_DT, NQ], fp8, tag="xt",
                                         name=f"x{t}{hl}{n}")
                              for n in range(4)]
        x3 = {k: v.rearrange("(t p) s -> p t s", p=128)
              for k, v in xdr.items()}

        def _xdma(eng, t, hl, n):
            eng.dma_start(xt[t + hl][n][:],
                          x3[t + "T" + hl][:, :, n * NQ:(n + 1) * NQ])

        def _wdma(eng, nm):
            eng.dma_start(w_sb[nm][:],
                          wdr[nm].rearrange("(t p) e -> p t e", p=128))

        # Everything on the SP queue: transfers serialize on the shared DMA
        # device regardless, and any DMA on the scalar queue would block the
        # ACT sequencer from issuing the (critical) exp stream.
        _wdma(nc.sync, "wkh")
        _wdma(nc.sync, "wkl")
        _xdma(nc.sync, "k", "h", 0)
        _xdma(nc.sync, "k", "l", 0)
        _wdma(nc.sync, "wvh")
        _wdma(nc.sync, "wvl")
        nc.sync.dma_start(ones_k[:], onesk)
        _xdma(nc.sync, "v", "h", 0)
        _xdma(nc.sync, "v", "l", 0)
        _wdma(nc.sync, "wqh")
        _wdma(nc.sync, "wql")
        _xdma(nc.sync, "q", "h", 0)
        _xdma(nc.sync, "q", "l", 0)
        nc.sync.dma_start(bq_sb[:], bq.rearrange("(m p) o -> p (m o)", p=128))
        for n in range(1, 4):
            _xdma(nc.sync, "v", "h", n)
            _xdma(nc.sync, "v", "l", n)
            _xdma(nc.sync, "k", "h", n)
            _xdma(nc.sync, "k", "l", n)
            _xdma(nc.sync, "q", "h", n)
            _xdma(nc.sync, "q", "l", n)
        nc.sync.dma_start(wo_sb[:], wo.rearrange("(t p) e -> p t e", p=128))

        # ---- PSUM pools: one shared 8-bank working set ------------------
        scp = ctx.enter_context(tc.tile_pool(name="scp", bufs=4, space="PSUM"))
        scr = ctx.enter_context(tc.tile_pool(name="scr", bufs=1, space="PSUM"))
        outp = ctx.enter_context(tc.tile_pool(name="outp", bufs=1, space="PSUM"))
        sump = ctx.enter_context(tc.tile_pool(name="sump", bufs=1, space="PSUM"))

        ptp = ctx.enter_context(tc.tile_pool(name="ptp", bufs=28))
        rp = ctx.enter_context(tc.tile_pool(name="rp", bufs=2))
        bcp = ctx.enter_context(tc.tile_pool(name="bcp", bufs=8))
        zsbp = ctx.enter_context(tc.tile_pool(name="zsbp", bufs=4))

        # ---- first K/Q n-chunk on the (still idle) score slots ----------
        # weights are host-scaled by 32 (fp8e4 subnormal avoidance); the
        # PSUM->SBUF copy applies the 1/32.  3 passes: wh@xh + wh@xl + wl@xh.
        def _dr_passes(t, wn):
            return ((w_sb[wn + "h"], xt[t + "h"]),
                    (w_sb[wn + "h"], xt[t + "l"]),
                    (w_sb[wn + "l"], xt[t + "h"]))

        def proj_big(t, wn, dst, n, bias):
            for m in range(2):
                ps = scp.tile([128, NQ], f32, tag="sc", name="projbig")
                passes = _dr_passes(t, wn)
                for pi, (wsb, xs) in enumerate(passes):
                    for dp in range(N_DT // 2):
                        nc.tensor.matmul(
                            ps[:], wsb[:, 2 * dp:2 * dp + 2,
                                       m * 128:(m + 1) * 128],
                            xs[n][:, 2 * dp:2 * dp + 2, :],
                            start=(pi == 0 and dp == 0),
                            stop=(pi == 2 and dp == N_DT // 2 - 1),
                            perf_mode=DR)
                if bias is None:
                    nc.vector.tensor_scalar_mul(
                        dst[m][:, n * NQ:(n + 1) * NQ], ps[:], 1.0 / 32)
                else:
                    nc.vector.tensor_scalar(
                        dst[m][:, n * NQ:(n + 1) * NQ], ps[:], 1.0 / 32,
                        bias[:, m:m + 1], ALU.mult, ALU.add)

        # V0-3 run during the kT/qT DMA wait and warm up the PE p-state
        # (they only need wv + the first vT chunk, which load first).
        # Dummy identity transposes (never read) fill the remaining DMA-wait
        # gaps so the p-state ramp reaches full speed before Kn0/Qn0.
        _V_PRE = 4
        wtp = scp.tile([128, 1024], fp16, tag="sc", name="wtp")

        def warm(cnt):
            for i in range(cnt):
                nc.tensor.matmul(
                    wtp[:, (i % 8) * 128:(i % 8 + 1) * 128], ident[:],
                    ident[:], is_transpose=True, start=True, stop=True,
                    skip_group_check=True)

        # ---- deferred projection backlog (drip-fed through scratch) -----
        def emit_v(st0, nst=1):
            vps = scr.tile([128, nst, E], f32, tag="scr", name="vps")
            passes = ((xt["vh"], w_sb["wvh"]), (xt["vl"], w_sb["wvh"]),
                      (xt["vh"], w_sb["wvl"]))
            for stl in range(nst):
                st = st0 + stl
                n, col = st // 4, (st % 4) * 128
                for pi, (xs, wsb) in enumerate(passes):
                    for dp in range(N_DT // 2):
                        nc.tensor.matmul(
                            vps[:, stl, :],
                            xs[n][:, 2 * dp:2 * dp + 2, col:col + 128],
                            wsb[:, 2 * dp:2 * dp + 2, :],
                            start=(pi == 0 and dp == 0 and stl == 0),
                            stop=(pi == 2 and dp == N_DT // 2 - 1),
                            perf_mode=DR, skip_group_check=True)
            nc.vector.tensor_scalar_mul(vh[:, st0:st0 + nst, :], vps[:],
                                        1.0 / 32)

        def emit_kq_chunk(t, wn, dst, n, m, bias, on_sc=False):
            if on_sc:
                ps = scp.tile([128, NQ], f32, tag="sc", name="kqps")
            else:
                ps = scr.tile([128, NQ], f32, tag="scr", name="kqps")
            passes = _dr_passes(t, wn)
            for pi, (wsb, xs) in enumerate(passes):
                for dp in range(N_DT // 2):
                    nc.tensor.matmul(
                        ps[:], wsb[:, 2 * dp:2 * dp + 2,
                                   m * 128:(m + 1) * 128],
                        xs[n][:, 2 * dp:2 * dp + 2, :],
                        start=(pi == 0 and dp == 0),
                        stop=(pi == 2 and dp == N_DT // 2 - 1),
                        perf_mode=DR)
            if bias is None:
                nc.vector.tensor_scalar_mul(
                    dst[m][:, n * NQ:(n + 1) * NQ], ps[:], 1.0 / 32)
            else:
                nc.vector.tensor_scalar(
                    dst[m][:, n * NQ:(n + 1) * NQ], ps[:], 1.0 / 32,
                    bias[:, m:m + 1], ALU.mult, ALU.add)

        warm(20)
        proj_big("k", "wk", kh, 0, None)
        warm(8)
        for st in range(_V_PRE):
            emit_v(st)
        warm(8)
        proj_big("q", "wq", qh, 0, bq_sb)

        backlog = []
        _K = lambda n, m, sc_=False: (
            lambda: emit_kq_chunk("k", "wk", kh, n, m, None, sc_))
        _Q = lambda n, m, sc_=False: (
            lambda: emit_kq_chunk("q", "wq", qh, n, m, bq_sb, sc_))
        _V = lambda st: (lambda: emit_v(st, 2))
        backlog += [_V(4), _K(1, 0, True), _K(1, 1), _V(6), _K(2, 0, True),
                    _K(2, 1), _V(8), _K(3, 0, True), _V(10), _K(3, 1),
                    _V(12), _Q(1, 0, True), _V(14), _Q(1, 1),
                    _Q(2, 0, True), _Q(2, 1), _Q(3, 0, True), _Q(3, 1)]
        backlog = backlog[::-1]  # pop() from the front

        # ---- attention + out_proj, per q-chunk --------------------------
        for qc in range(N_QC):
            q0, q1 = qc * NQ, (qc + 1) * NQ
            out2 = outp.tile([128, 4, E], f32, tag="out2", name="out2")
            sums = sump.tile([128, 16], f32, tag="sums", name="sums")

            def pv_sums(kt, pts):
                # only the FIRST matmul touching each PSUM bank of an
                # accumulation group may set start=True (bank-granular zero)
                for h in range(4):
                    for qt in range(4):
                        lhsT = pts[h][:, qt * 128:(qt + 1) * 128]
                        nc.tensor.matmul(
                            out2[:, qt, h * 64:(h + 1) * 64], lhsT,
                            vh[:, kt, h * 64:(h + 1) * 64],
                            start=(kt == 0 and h == 0 and qt % 2 == 0),
                            stop=(kt == N_KT - 1),
                            skip_group_check=True)
                        nc.tensor.matmul(
                            sums[:, qt * 4 + h:qt * 4 + h + 1], lhsT,
                            ones_k[:],
                            start=(kt == 0 and h == 0 and qt == 0),
                            stop=(kt == N_KT - 1),
                            skip_group_check=True)

            draw = _DRAW.get(qc, [0] * N_KT)
            prev_pts = None
            for kt in range(N_KT):
                k0 = kt * 128
                scs = []
                for h in range(4):
                    p, j = h // 2, h % 2
                    lo, hi = j * 64, (j + 1) * 64
                    sc = scp.tile([128, NQ], f32, tag="sc", name="sc")
                    nc.tensor.matmul(
                        sc[:], kh[p][lo:hi, k0:k0 + 128],
                        qh[p][lo:hi, q0:q1], start=True, stop=True)
                    scs.append(sc)
                if prev_pts is not None:
                    pv_sums(kt - 1, prev_pts)
                for _ in range(draw[kt]):
                    if backlog:
                        backlog.pop()()
                pts = []
                for h in range(4):
                    off = ((kt * 4 + h) % 16 == 9 if qc == 0
                           else (kt * 4 + h) % 8 in (1, 4, 6))
                    if off:
                        pti = ptp.tile([128, NQ], i16, tag="pt", name="pti")
                        nc.vector.tensor_scalar(
                            pti[:], scs[h][:], SCH_A, SCH_B,
                            ALU.mult, ALU.add)
                        pt = pti.bitcast(fp16)
                    else:
                        pt = ptp.tile([128, NQ], fp16, tag="pt", name="pt")
                        nc.scalar.activation(pt[:], scs[h][:], AF.Exp,
                                             scale=0.125)
                    pts.append(pt)
                prev_pts = pts
            pv_sums(N_KT - 1, prev_pts)

            # drain: normalize in [q, e], transpose to [e, q], out_proj.
            # All 16 normalize blocks go first (DVE/ACT alternating on the
            # last chunk), then all transposes, then all ot copies, so the
            # engines pipeline instead of chaining.  The out_proj of qc<3 is
            # deferred into the NEXT q-chunk's backlog (so it never blocks
            # the loop); qc3's runs on the freed score slots.
            last = qc == N_QC - 1
            rv = rp.tile([128, 16], f32, tag="rv", name="rv")
            nc.vector.reciprocal(rv[:], sums[:])
            o2n = []
            for qt in range(4):
                o2 = bcp.tile([128, E], fp16, tag="o2n", name="o2n")
                o2n.append(o2)
            # block-split DVE/ACT on the last chunk (qt0/1 vs qt2/3) so
            # each engine's ops chain densely instead of ping-ponging
            for qt in range(4):
                for h in range(4):
                    c0 = qt * 4 + h
                    if last and qt >= 2:
                        nc.scalar.activation(
                            o2n[qt][:, h * 64:(h + 1) * 64],
                            out2[:, qt, h * 64:(h + 1) * 64],
                            AF.Copy, scale=rv[:, c0:c0 + 1])
                    else:
                        nc.vector.tensor_scalar_mul(
                            o2n[qt][:, h * 64:(h + 1) * 64],
                            out2[:, qt, h * 64:(h + 1) * 64],
                            rv[:, c0:c0 + 1])
            tp = scr.tile([128, 1024], fp16, tag="scr", name="tp")
            for qt in range(4):
                for et in range(2):
                    blk = qt * 2 + et
                    nc.tensor.matmul(
                        tp[:, blk * 128:(blk + 1) * 128],
                        o2n[qt][:, et * 128:(et + 1) * 128], ident[:],
                        is_transpose=True, start=True, stop=True,
                        skip_group_check=True)
            for qt in range(4):
                for et in range(2):
                    blk = qt * 2 + et
                    if last and qt >= 2:
                        nc.scalar.activation(
                            ot[et][:, q0 + qt * 128:q0 + (qt + 1) * 128],
                            tp[:, blk * 128:(blk + 1) * 128],
                            AF.Copy, scale=1.0)
                    else:
                        nc.vector.tensor_copy(
                            ot[et][:, q0 + qt * 128:q0 + (qt + 1) * 128],
                            tp[:, blk * 128:(blk + 1) * 128])

            def out_proj_chunk(qc_, e, on_sc):
                q0_, q1_ = qc_ * NQ, (qc_ + 1) * NQ
                if on_sc:
                    zps = scp.tile([128, NQ], f32, tag="sc", name="zps")
                else:
                    zps = scr.tile([128, NQ], f32, tag="scr", name="zps")
                for c in range(2):
                    nc.tensor.matmul(
                        zps[:], wo_sb[:, c, e * 128:(e + 1) * 128],
                        ot[c][:, q0_:q1_], start=(c == 0), stop=(c == 1))
                zsb = zsbp.tile([128, NQ], fp16, tag="zsb", name="zsb")
                if on_sc and e >= 4:
                    nc.scalar.activation(zsb[:], zps[:], AF.Copy, scale=1.0)
                else:
                    nc.vector.tensor_copy(zsb[:], zps[:])
                nc.sync.dma_start(zT[e * 128:(e + 1) * 128, q0_:q1_], zsb[:])

            if last:
                for e in range(N_DT):
                    out_proj_chunk(qc, e, True)
            else:
                for e in range(N_DT):
                    out_proj_chunk(qc, e, False)

    nc.compile()
    return nc


def _get_program():
    global _PROGRAM
    if _PROGRAM is None:
        _PROGRAM = _build_program()
    return _PROGRAM


ONESK_NP = None


def _init_consts():
    global ONESK_NP
    if ONESK_NP is None:
        ONESK_NP = np.ones((128, 1), np.float16)


def _hilo(a, f8):
    hi = a.astype(f8)
    lo = (a - hi.astype(np.float32)).astype(f8)
    return np.ascontiguousarray(hi), np.ascontiguousarray(lo)


def _make_in_maps(q, k, v, Wq, bq, Wk, Wv, Wo):
    _init_consts()
    import ml_dtypes
    f8 = ml_dtypes.float8_e4m3
    f32 = np.float32
    xT = {}
    for b in range(B):
        for nm, x in (("q", q), ("k", k), ("v", v)):
            h, lo = _hilo(np.ascontiguousarray(x[b].T), f8)
            xT[(nm, b)] = (h, lo)
    wslices = {}
    for g in range(4):
        sl = slice(g * E, (g + 1) * E)
        for nm, W in (("wq", Wq), ("wk", Wk), ("wv", Wv)):
            h, lo = _hilo(np.ascontiguousarray(W[sl, :].T) * 32.0, f8)
            wslices[(nm, g)] = (h, lo)
        wslices[("wo", g)] = np.ascontiguousarray(Wo[:, sl].T, dtype=np.float16)
        wslices[("bq", g)] = np.ascontiguousarray(
            bq[sl].reshape(E, 1), dtype=f32)
    in_maps = []
    for c in range(N_CORES):
        b, g = c // 4, c % 4
        in_maps.append({
            "onesk": ONESK_NP,
            "qTh": xT[("q", b)][0], "qTl": xT[("q", b)][1],
            "kTh": xT[("k", b)][0], "kTl": xT[("k", b)][1],
            "vTh": xT[("v", b)][0], "vTl": xT[("v", b)][1],
            "wqh": wslices[("wq", g)][0], "wql": wslices[("wq", g)][1],
            "wkh": wslices[("wk", g)][0], "wkl": wslices[("wk", g)][1],
            "wvh": wslices[("wv", g)][0], "wvl": wslices[("wv", g)][1],
            "wo": wslices[("wo", g)], "bq": wslices[("bq", g)],
        })
    return in_maps


def _numpy_fallback(q, k, v, mask, Wq, bq, Wk, bk, Wv, bv, Wo, bo):
    # Only used if mask is not all-True (never the case for this problem).
    def proj(x, W, b_):
        y = x @ W.T + b_
        return y.reshape(B, S, NUM_HEADS, DK).transpose(0, 2, 1, 3)
    qh, kh, vh = proj(q, Wq, bq), proj(k, Wk, bk), proj(v, Wv, bv)
    sc = np.einsum("bhqd,bhkd->bhqk", qh, kh) / np.sqrt(DK)
    sc = np.where(mask, sc, np.float32(-1e9))
    sc = sc - sc.max(-1, keepdims=True)
    p = np.exp(sc)
    p /= p.sum(-1, keepdims=True)
    o = np.einsum("bhqk,bhkd->bhqd", p, vh)
    o = o.transpose(0, 2, 1, 3).reshape(B, S, D_MODEL)
    return (o @ Wo.T + bo).astype(np.float32)


def kernel(q, k, v, mask, Wq, bq, Wk, bk, Wv, bv, Wo, bo):
    q = np.asarray(q, dtype=np.float32)
    k = np.asarray(k, dtype=np.float32)
    v = np.asarray(v, dtype=np.float32)
    Wq, Wk, Wv, Wo = (np.asarray(w, dtype=np.float32) for w in (Wq, Wk, Wv, Wo))
    bq, bk, bv, bo = (np.asarray(x, dtype=np.float32) for x in (bq, bk, bv, bo))
    if not np.all(np.asarray(mask)):
        return _numpy_fallback(q, k, v, np.asarray(mask), Wq, bq, Wk, bk,
                               Wv, bv, Wo, bo)

    from concourse.bass_utils import run_bass_kernel_spmd
    nc = _get_program()
    in_maps = _make_in_maps(q, k, v, Wq, bq, Wk, Wv, Wo)
    res = run_bass_kernel_spmd(nc, in_maps, core_ids=list(range(N_CORES)),
                               **_RUN_KWARGS)
    global _LAST_RESULTS
    _LAST_RESULTS = res
    # bk is dropped on-device (exact: softmax shift invariance); bv is
    # folded into the output bias (attention weights sum to 1).
    bo_eff = bo + Wo @ bv
    out = np.empty((B, S, D_MODEL), dtype=np.float32)
    for b in range(B):
        acc = res.results[4 * b]["zT"].astype(np.float32)
        for g in range(1, 4):
            acc = acc + res.results[4 * b + g]["zT"].astype(np.float32)
        out[b] = acc.T + bo_eff
    return out


# revision 33
# speedup vs baseline: 1.0050x; 1.0005x over previous
"""Trainium2 Bass kernel for nn_MultiHeadAttention (B=2, S=2048, d_model=1024, H=16).

Sharding (8 cores): data-parallel over B (2) x tensor-parallel over head groups
(4 groups of 4 heads).  Each core computes its head-group's Q/K/V projections
(column-sharded weights), attention for its 4 heads, and a row-parallel
out_proj partial product.  The host sums the 4 partials per batch (the
"all-reduce") and adds the output bias.

v3 design notes (cost-model driven):
  - P@V uses SWAPPED operands: P (exp scores, [k,q]) stationary, V ([k,e])
    moving, so the moving free dim is 64 instead of 512; attention output
    lands in [q, e].  Softmax denominators ride along as N=1 matmuls
    (rhs = ones) reusing the loaded P stationary tile.
  - Normalization = per-partition scalar multiply on DVE; PE transposes
    bring [q, e] back to [e, q] for the row-parallel out_proj.
  - PSUM is a single 8-bank working set shared by EVERYTHING (no stacked
    stage pools, which would serialize projections before attention):
    sc 2x2 banks, out2 2, sums 1, scratch 1.  Projections beyond the
    first k/q n-chunk are drip-fed through the scratch bank inside the
    attention loop (deadline-ordered backlog), so the ACT exp stream --
    the critical resource -- starts ~15us in instead of ~65us.
  - x is loaded in [128, d, 512] n-chunks (one DMA each) so the first
    chunk of K and Q arrives after ~9us of serial DMA instead of ~30us.
  - bk is dropped exactly (softmax shift invariance); bv is folded into
    the host-side output bias (attention weights sum to 1); bq is applied
    on-device during the qh PSUM->SBUF copy.
  - PSUM start_tensor_calc zeroing is bank-granular: only the first
    matmul touching a bank in an accumulation group sets start=True.
"""

import sys
import numpy as np

for _p in ("/opt/trn_rl_repo", "/root/.axon_site/_ro/trn_rl_repo"):
    if _p not in sys.path:
        sys.path.append(_p)

D_MODEL = 1024
NUM_HEADS = 16
DK = 64
B = 2
S = 2048
N_CORES = 8
HPC = 4               # heads per core
E = HPC * DK          # 256 features per core
NQ = 512              # q-chunk size
N_QC = S // NQ        # 4 q chunks
N_KT = S // 128       # 16 k tiles
N_DT = D_MODEL // 128  # 8 contraction tiles for projections

_PROGRAM = None
_RUN_KWARGS = {}      # test harness may set {"trace": True}
_LAST_RESULTS = None  # BassKernelResults of the last run

# Backlog draw schedule: how many deferred projection chunks to emit
# after each (qc, kt) iteration of the attention loop.
_DRAW = {0: [1, 1, 1, 1, 1, 1, 1, 1, 1, 1, 1, 1, 1, 1, 0, 0],
         1: [1, 1, 1, 1] + [0] * 12}


def _build_program():
    import concourse.bass as bass
    import concourse.mybir as mybir
    from concourse import bacc, tile
    from contextlib import ExitStack

    f32 = mybir.dt.float32
    fp16 = mybir.dt.float16
    i16 = mybir.dt.int16
    AF = mybir.ActivationFunctionType
    ALU = mybir.AluOpType
    # Schraudolph fast-exp constants (int16/fp16 bitcast):
    #   i16 = round(s * 0.125 * 1024/ln2 + (15*1024 - C));  C tuned for
    #   min RMS rel error (~1.8%); applied to ~22% of exp tiles on DVE.
    SCH_A = 0.125 * 1024.0 / np.log(2.0)
    SCH_B = 15.0 * 1024.0 - 60.0

    nc = bacc.Bacc("TRN2", target_bir_lowering=False, debug=False,
                   num_devices=N_CORES)

    fp8 = mybir.dt.float8e4
    DR = mybir.MatmulPerfMode.DoubleRow
    xdr = {}
    for nm in ("qTh", "qTl", "kTh", "kTl", "vTh", "vTl"):
        xdr[nm] = nc.dram_tensor(nm, [D_MODEL, S], fp8,
                                 kind="ExternalInput").ap()
    wdr = {}
    for nm in ("wqh", "wql", "wkh", "wkl", "wvh", "wvl"):
        wdr[nm] = nc.dram_tensor(nm, [D_MODEL, E], fp8,
                                 kind="ExternalInput").ap()
    wo = nc.dram_tensor("wo", [E, D_MODEL], fp16, kind="ExternalInput").ap()
    bq = nc.dram_tensor("bq", [E, 1], f32, kind="ExternalInput").ap()
    onesk = nc.dram_tensor("onesk", [128, 1], fp16, kind="ExternalInput").ap()
    zT = nc.dram_tensor("zT", [D_MODEL, S], fp16, kind="ExternalOutput").ap()

    with tile.TileContext(nc) as tc, ExitStack() as ctx:
        persist = ctx.enter_context(tc.tile_pool(name="persist", bufs=1))
        const = ctx.enter_context(tc.tile_pool(name="const", bufs=1))

        w_sb = {}
        for nm in ("wvh", "wvl", "wkh", "wkl", "wqh", "wql"):
            w_sb[nm] = persist.tile([128, N_DT, E], fp8, tag=nm, name=nm)
        wo_sb = persist.tile([128, 2, D_MODEL], fp16, tag="wo", name="wo")
        bq_sb = persist.tile([128, 2], f32, tag="bq", name="bq")

        from concourse.masks import make_identity
        ident = const.tile([128, 128], fp16, tag="ident", name="ident")
        make_identity(nc, ident)
        ones_k = const.tile([128, 1], fp16, tag="ones_k", name="ones_k")

        qh = [persist.tile([128, S], fp16, tag=f"qh{p}", name=f"qh{p}")
              for p in range(2)]
        kh = [persist.tile([128, S], fp16, tag=f"kh{p}", name=f"kh{p}")
              for p in range(2)]
        vh = persist.tile([128, N_KT, E], fp16, tag="vh", name="vh")
        ot = [persist.tile([128, S], fp16, tag=f"ot{p}", name=f"ot{p}")
              for p in range(2)]

        # ---- x chunk tiles + DMA schedule (priority order) --------------
        xpool = ctx.enter_context(tc.tile_pool(name="xpool", bufs=24))
        xt = {}
        for t in ("k", "q", "v"):
            for hl in "hl":
                xt[t + hl] = [xpool.tile([128, N_DT, NQ], fp8, tag="xt",
                                         name=f"x{t}{hl}{n}")
                              for n in range(4)]
        x3 = {k: v.rearrange("(t p) s -> p t s", p=128)
              for k, v in xdr.items()}

        def _xdma(eng, t, hl, n):
            eng.dma_start(xt[t + hl][n][:],
                          x3[t + "T" + hl][:, :, n * NQ:(n + 1) * NQ])

        def _wdma(eng, nm):
            eng.dma_start(w_sb[nm][:],
                          wdr[nm].rearrange("(t p) e -> p t e", p=128))

        # Everything on the SP queue: transfers serialize on the shared DMA
        # device regardless, and any DMA on the scalar queue would block the
        # ACT sequencer from issuing the (critical) exp stream.
        _wdma(nc.sync, "wkh")
        _wdma(nc.sync, "wkl")
        _xdma(nc.sync, "k", "h", 0)
        _xdma(nc.sync, "k", "l", 0)
        _wdma(nc.sync, "wvh")
        _wdma(nc.sync, "wvl")
        nc.sync.dma_start(ones_k[:], onesk)
        _xdma(nc.sync, "v", "h", 0)
        _xdma(nc.sync, "v", "l", 0)
        _wdma(nc.sync, "wqh")
        _wdma(nc.sync, "wql")
        _xdma(nc.sync, "q", "h", 0)
        _xdma(nc.sync, "q", "l", 0)
        nc.sync.dma_start(bq_sb[:], bq.rearrange("(m p) o -> p (m o)", p=128))
        for n in range(1, 4):
            _xdma(nc.sync, "v", "h", n)
            _xdma(nc.sync, "v", "l", n)
            _xdma(nc.sync, "k", "h", n)
            _xdma(nc.sync, "k", "l", n)
            _xdma(nc.sync, "q", "h", n)
            _xdma(nc.sync, "q", "l", n)
        nc.sync.dma_start(wo_sb[:], wo.rearrange("(t p) e -> p t e", p=128))

        # ---- PSUM pools: one shared 8-bank working set ------------------
        scp = ctx.enter_context(tc.tile_pool(name="scp", bufs=4, space="PSUM"))
        scr = ctx.enter_context(tc.tile_pool(name="scr", bufs=1, space="PSUM"))
        outp = ctx.enter_context(tc.tile_pool(name="outp", bufs=1, space="PSUM"))
        sump = ctx.enter_context(tc.tile_pool(name="sump", bufs=1, space="PSUM"))

        ptp = ctx.enter_context(tc.tile_pool(name="ptp", bufs=28))
        rp = ctx.enter_context(tc.tile_pool(name="rp", bufs=2))
        bcp = ctx.enter_context(tc.tile_pool(name="bcp", bufs=8))
        zsbp = ctx.enter_context(tc.tile_pool(name="zsbp", bufs=4))

        # ---- first K/Q n-chunk on the (still idle) score slots ----------
        # weights are host-scaled by 32 (fp8e4 subnormal avoidance); the
        # PSUM->SBUF copy applies the 1/32.  3 passes: wh@xh + wh@xl + wl@xh.
        def _dr_passes(t, wn):
            return ((w_sb[wn + "h"], xt[t + "h"]),
                    (w_sb[wn + "h"], xt[t + "l"]),
                    (w_sb[wn + "l"], xt[t + "h"]))

        def proj_big(t, wn, dst, n, bias):
            for m in range(2):
                ps = scp.tile([128, NQ], f32, tag="sc", name="projbig")
                passes = _dr_passes(t, wn)
                for pi, (wsb, xs) in enumerate(passes):
                    for dp in range(N_DT // 2):
                        nc.tensor.matmul(
                            ps[:], wsb[:, 2 * dp:2 * dp + 2,
                                       m * 128:(m + 1) * 128],
                            xs[n][:, 2 * dp:2 * dp + 2, :],
                            start=(pi == 0 and dp == 0),
                            stop=(pi == 2 and dp == N_DT // 2 - 1),
                            perf_mode=DR)
                if bias is None:
                    nc.vector.tensor_scalar_mul(
                        dst[m][:, n * NQ:(n + 1) * NQ], ps[:], 1.0 / 32)
                else:
                    nc.vector.tensor_scalar(
                        dst[m][:, n * NQ:(n + 1) * NQ], ps[:], 1.0 / 32,
                        bias[:, m:m + 1], ALU.mult, ALU.add)

        # V0-3 run during the kT/qT DMA wait and warm up the PE p-state
        # (they only need wv + the first vT chunk, which load first).
        # Dummy identity transposes (never read) fill the remaining DMA-wait
        # gaps so the p-state ramp reaches full speed before Kn0/Qn0.
        _V_PRE = 4
        wtp = scp.tile([128, 1024], fp16, tag="sc", name="wtp")

        def warm(cnt):
            for i in range(cnt):
                nc.tensor.matmul(
                    wtp[:, (i % 8) * 128:(i % 8 + 1) * 128], ident[:],
                    ident[:], is_transpose=True, start=True, stop=True,
                    skip_group_check=True)

        # ---- deferred projection backlog (drip-fed through scratch) -----
        def emit_v(st0, nst=1):
            vps = scr.tile([128, nst, E], f32, tag="scr", name="vps")
            passes = ((xt["vh"], w_sb["wvh"]), (xt["vl"], w_sb["wvh"]),
                      (xt["vh"], w_sb["wvl"]))
            for stl in range(nst):
                st = st0 + stl
                n, col = st // 4, (st % 4) * 128
                for pi, (xs, wsb) in enumerate(passes):
                    for dp in range(N_DT // 2):
                        nc.tensor.matmul(
                            vps[:, stl, :],
                            xs[n][:, 2 * dp:2 * dp + 2, col:col + 128],
                            wsb[:, 2 * dp:2 * dp + 2, :],
                            start=(pi == 0 and dp == 0 and stl == 0),
                            stop=(pi == 2 and dp == N_DT // 2 - 1),
                            perf_mode=DR, skip_group_check=True)
            nc.vector.tensor_scalar_mul(vh[:, st0:st0 + nst, :], vps[:],
                                        1.0 / 32)

        def emit_kq_chunk(t, wn, dst, n, m, bias, on_sc=False):
            if on_sc:
                ps = scp.tile([128, NQ], f32, tag="sc", name="kqps")
            else:
                ps = scr.tile([128, NQ], f32, tag="scr", name="kqps")
            passes = _dr_passes(t, wn)
            for pi, (wsb, xs) in enumerate(passes):
                for dp in range(N_DT // 2):
                    nc.tensor.matmul(
                        ps[:], wsb[:, 2 * dp:2 * dp + 2,
                                   m * 128:(m + 1) * 128],
                        xs[n][:, 2 * dp:2 * dp + 2, :],
                        start=(pi == 0 and dp == 0),
                        stop=(pi == 2 and dp == N_DT // 2 - 1),
                        perf_mode=DR)
            if bias is None:
                nc.vector.tensor_scalar_mul(
                    dst[m][:, n * NQ:(n + 1) * NQ], ps[:], 1.0 / 32)
            else:
                nc.vector.tensor_scalar(
                    dst[m][:, n * NQ:(n + 1) * NQ], ps[:], 1.0 / 32,
                    bias[:, m:m + 1], ALU.mult, ALU.add)

        warm(20)
        proj_big("k", "wk", kh, 0, None)
        warm(8)
        for st in range(_V_PRE):
            emit_v(st)
        warm(8)
        proj_big("q", "wq", qh, 0, bq_sb)

        backlog = []
        _K = lambda n, m, sc_=False: (
            lambda: emit_kq_chunk("k", "wk", kh, n, m, None, sc_))
        _Q = lambda n, m, sc_=False: (
            lambda: emit_kq_chunk("q", "wq", qh, n, m, bq_sb, sc_))
        _V = lambda st: (lambda: emit_v(st, 2))
        backlog += [_V(4), _K(1, 0, True), _K(1, 1, True), _V(6),
                    _K(2, 0, True), _K(2, 1, True), _V(8), _K(3, 0, True),
                    _V(10), _K(3, 1, True), _V(12), _Q(1, 0, True), _V(14),
                    _Q(1, 1, True), _Q(2, 0, True), _Q(2, 1, True),
                    _Q(3, 0, True), _Q(3, 1, True)]
        backlog = backlog[::-1]  # pop() from the front

        # ---- attention + out_proj, per q-chunk --------------------------
        for qc in range(N_QC):
            q0, q1 = qc * NQ, (qc + 1) * NQ
            out2 = outp.tile([128, 4, E], f32, tag="out2", name="out2")
            sums = sump.tile([128, 16], f32, tag="sums", name="sums")

            def pv_sums(kt, pts):
                # only the FIRST matmul touching each PSUM bank of an
                # accumulation group may set start=True (bank-granular zero)
                for h in range(4):
                    for qt in range(4):
                        lhsT = pts[h][:, qt * 128:(qt + 1) * 128]
                        nc.tensor.matmul(
                            out2[:, qt, h * 64:(h + 1) * 64], lhsT,
                            vh[:, kt, h * 64:(h + 1) * 64],
                            start=(kt == 0 and h == 0 and qt % 2 == 0),
                            stop=(kt == N_KT - 1),
                            skip_group_check=True)
                        nc.tensor.matmul(
                            sums[:, qt * 4 + h:qt * 4 + h + 1], lhsT,
                            ones_k[:],
                            start=(kt == 0 and h == 0 and qt == 0),
                            stop=(kt == N_KT - 1),
                            skip_group_check=True)

            draw = _DRAW.get(qc, [0] * N_KT)
            prev_pts = None
            for kt in range(N_KT):
                k0 = kt * 128
                scs = []
                for h in range(4):
                    p, j = h // 2, h % 2
                    lo, hi = j * 64, (j + 1) * 64
                    sc = scp.tile([128, NQ], f32, tag="sc", name="sc")
                    nc.tensor.matmul(
                        sc[:], kh[p][lo:hi, k0:k0 + 128],
                        qh[p][lo:hi, q0:q1], start=True, stop=True)
                    scs.append(sc)
                if prev_pts is not None:
                    pv_sums(kt - 1, prev_pts)
                for _ in range(draw[kt]):
                    if backlog:
                        backlog.pop()()
                pts = []
                for h in range(4):
                    off = ((kt * 4 + h) % 16 == 9 if qc == 0
                           else (kt * 4 + h) % 8 in (1, 4, 6))
                    if off:
                        pti = ptp.tile([128, NQ], i16, tag="pt", name="pti")
                        nc.vector.tensor_scalar(
                            pti[:], scs[h][:], SCH_A, SCH_B,
                            ALU.mult, ALU.add)
                        pt = pti.bitcast(fp16)
                    else:
                        pt = ptp.tile([128, NQ], fp16, tag="pt", name="pt")
                        nc.scalar.activation(pt[:], scs[h][:], AF.Exp,
                                             scale=0.125)
                    pts.append(pt)
                prev_pts = pts
            pv_sums(N_KT - 1, prev_pts)

            # drain: normalize in [q, e], transpose to [e, q], out_proj.
            # All 16 normalize blocks go first (DVE/ACT alternating on the
            # last chunk), then all transposes, then all ot copies, so the
            # engines pipeline instead of chaining.  The out_proj of qc<3 is
            # deferred into the NEXT q-chunk's backlog (so it never blocks
            # the loop); qc3's runs on the freed score slots.
            last = qc == N_QC - 1
            rv = rp.tile([128, 16], f32, tag="rv", name="rv")
            nc.vector.reciprocal(rv[:], sums[:])
            o2n = []
            for qt in range(4):
                o2 = bcp.tile([128, E], fp16, tag="o2n", name="o2n")
                o2n.append(o2)
            # block-split DVE/ACT on the last chunk (qt0/1 vs qt2/3) so
            # each engine's ops chain densely instead of ping-ponging
            for qt in range(4):
                for h in range(4):
                    c0 = qt * 4 + h
                    if last and qt >= 2:
                        nc.scalar.activation(
                            o2n[qt][:, h * 64:(h + 1) * 64],
                            out2[:, qt, h * 64:(h + 1) * 64],
                            AF.Copy, scale=rv[:, c0:c0 + 1])
                    else:
                        nc.vector.tensor_scalar_mul(
                            o2n[qt][:, h * 64:(h + 1) * 64],
                            out2[:, qt, h * 64:(h + 1) * 64],
                            rv[:, c0:c0 + 1])
            tp = scr.tile([128, 1024], fp16, tag="scr", name="tp")
            for qt in range(4):
                for et in range(2):
                    blk = qt * 2 + et
                    nc.tensor.matmul(
                        tp[:, blk * 128:(blk + 1) * 128],
                        o2n[qt][:, et * 128:(et + 1) * 128], ident[:],
                        is_transpose=True, start=True, stop=True,
                        skip_group_check=True)
            for qt in range(4):
                for et in range(2):
                    blk = qt * 2 + et
                    if last and qt >= 2:
                        nc.scalar.activation(
                            ot[et][:, q0 + qt * 128:q0 + (qt + 1) * 128],
                            tp[:, blk * 128:(blk + 1) * 128],
                            AF.Copy, scale=1.0)
                    else:
                        nc.vector.tensor_copy(
                            ot[et][:, q0 + qt * 128:q0 + (qt + 1) * 128],
                            tp[:, blk * 128:(blk + 1) * 128])

            def out_proj_chunk(qc_, e, on_sc):
                q0_, q1_ = qc_ * NQ, (qc_ + 1) * NQ
                if on_sc:
                    zps = scp.tile([128, NQ], f32, tag="sc", name="zps")
                else:
                    zps = scr.tile([128, NQ], f32, tag="scr", name="zps")
                for c in range(2):
                    nc.tensor.matmul(
                        zps[:], wo_sb[:, c, e * 128:(e + 1) * 128],
                        ot[c][:, q0_:q1_], start=(c == 0), stop=(c == 1))
                zsb = zsbp.tile([128, NQ], fp16, tag="zsb", name="zsb")
                if on_sc and e >= 4:
                    nc.scalar.activation(zsb[:], zps[:], AF.Copy, scale=1.0)
                else:
                    nc.vector.tensor_copy(zsb[:], zps[:])
                nc.sync.dma_start(zT[e * 128:(e + 1) * 128, q0_:q1_], zsb[:])

            if last:
                for e in range(N_DT):
                    out_proj_chunk(qc, e, True)
            else:
                for e in range(N_DT):
                    out_proj_chunk(qc, e, False)

    nc.compile()
    return nc


def _get_program():
    global _PROGRAM
    if _PROGRAM is None:
        _PROGRAM = _build_program()
    return _PROGRAM


ONESK_NP = None


def _init_consts():
    global ONESK_NP
    if ONESK_NP is None:
        ONESK_NP = np.ones((128, 1), np.float16)


def _hilo(a, f8):
    hi = a.astype(f8)
    lo = (a - hi.astype(np.float32)).astype(f8)
    return np.ascontiguousarray(hi), np.ascontiguousarray(lo)


def _make_in_maps(q, k, v, Wq, bq, Wk, Wv, Wo):
    _init_consts()
    import ml_dtypes
    f8 = ml_dtypes.float8_e4m3
    f32 = np.float32
    xT = {}
    for b in range(B):
        for nm, x in (("q", q), ("k", k), ("v", v)):
            h, lo = _hilo(np.ascontiguousarray(x[b].T), f8)
            xT[(nm, b)] = (h, lo)
    wslices = {}
    for g in range(4):
        sl = slice(g * E, (g + 1) * E)
        for nm, W in (("wq", Wq), ("wk", Wk), ("wv", Wv)):
            h, lo = _hilo(np.ascontiguousarray(W[sl, :].T) * 32.0, f8)
            wslices[(nm, g)] = (h, lo)
        wslices[("wo", g)] = np.ascontiguousarray(Wo[:, sl].T, dtype=np.float16)
        wslices[("bq", g)] = np.ascontiguousarray(
            bq[sl].reshape(E, 1), dtype=f32)
    in_maps = []
    for c in range(N_CORES):
        b, g = c // 4, c % 4
        in_maps.append({
            "onesk": ONESK_NP,
            "qTh": xT[("q", b)][0], "qTl": xT[("q", b)][1],
            "kTh": xT[("k", b)][0], "kTl": xT[("k", b)][1],
            "vTh": xT[("v", b)][0], "vTl": xT[("v", b)][1],
            "wqh": wslices[("wq", g)][0], "wql": wslices[("wq", g)][1],
            "wkh": wslices[("wk", g)][0], "wkl": wslices[("wk", g)][1],
            "wvh": wslices[("wv", g)][0], "wvl": wslices[("wv", g)][1],
            "wo": wslices[("wo", g)], "bq": wslices[("bq", g)],
        })
    return in_maps


def _numpy_fallback(q, k, v, mask, Wq, bq, Wk, bk, Wv, bv, Wo, bo):
    # Only used if mask is not all-True (never the case for this problem).
    def proj(x, W, b_):
        y = x @ W.T + b_
        return y.reshape(B, S, NUM_HEADS, DK).transpose(0, 2, 1, 3)
    qh, kh, vh = proj(q, Wq, bq), proj(k, Wk, bk), proj(v, Wv, bv)
    sc = np.einsum("bhqd,bhkd->bhqk", qh, kh) / np.sqrt(DK)
    sc = np.where(mask, sc, np.float32(-1e9))
    sc = sc - sc.max(-1, keepdims=True)
    p = np.exp(sc)
    p /= p.sum(-1, keepdims=True)
    o = np.einsum("bhqk,bhkd->bhqd", p, vh)
    o = o.transpose(0, 2, 1, 3).reshape(B, S, D_MODEL)
    return (o @ Wo.T + bo).astype(np.float32)


def kernel(q, k, v, mask, Wq, bq, Wk, bk, Wv, bv, Wo, bo):
    q = np.asarray(q, dtype=np.float32)
    k = np.asarray(k, dtype=np.float32)
    v = np.asarray(v, dtype=np.float32)
    Wq, Wk, Wv, Wo = (np.asarray(w, dtype=np.float32) for w in (Wq, Wk, Wv, Wo))
    bq, bk, bv, bo = (np.asarray(x, dtype=np.float32) for x in (bq, bk, bv, bo))
    if not np.all(np.asarray(mask)):
        return _numpy_fallback(q, k, v, np.asarray(mask), Wq, bq, Wk, bk,
                               Wv, bv, Wo, bo)

    from concourse.bass_utils import run_bass_kernel_spmd
    nc = _get_program()
    in_maps = _make_in_maps(q, k, v, Wq, bq, Wk, Wv, Wo)
    res = run_bass_kernel_spmd(nc, in_maps, core_ids=list(range(N_CORES)),
                               **_RUN_KWARGS)
    global _LAST_RESULTS
    _LAST_RESULTS = res
    # bk is dropped on-device (exact: softmax shift invariance); bv is
    # folded into the output bias (attention weights sum to 1).
    bo_eff = bo + Wo @ bv
    out = np.empty((B, S, D_MODEL), dtype=np.float32)
    for b in range(B):
        acc = res.results[4 * b]["zT"].astype(np.float32)
        for g in range(1, 4):
            acc = acc + res.results[4 * b + g]["zT"].astype(np.float32)
        out[b] = acc.T + bo_eff
    return out


# revision 38
# speedup vs baseline: 1.0091x; 1.0040x over previous
"""Trainium2 Bass kernel for nn_MultiHeadAttention (B=2, S=2048, d_model=1024, H=16).

Sharding (8 cores): data-parallel over B (2) x tensor-parallel over head groups
(4 groups of 4 heads).  Each core computes its head-group's Q/K/V projections
(column-sharded weights), attention for its 4 heads, and a row-parallel
out_proj partial product.  The host sums the 4 partials per batch (the
"all-reduce") and adds the output bias.

v3 design notes (cost-model driven):
  - P@V uses SWAPPED operands: P (exp scores, [k,q]) stationary, V ([k,e])
    moving, so the moving free dim is 64 instead of 512; attention output
    lands in [q, e].  Softmax denominators ride along as N=1 matmuls
    (rhs = ones) reusing the loaded P stationary tile.
  - Normalization = per-partition scalar multiply on DVE; PE transposes
    bring [q, e] back to [e, q] for the row-parallel out_proj.
  - PSUM is a single 8-bank working set shared by EVERYTHING (no stacked
    stage pools, which would serialize projections before attention):
    sc 2x2 banks, out2 2, sums 1, scratch 1.  Projections beyond the
    first k/q n-chunk are drip-fed through the scratch bank inside the
    attention loop (deadline-ordered backlog), so the ACT exp stream --
    the critical resource -- starts ~15us in instead of ~65us.
  - x is loaded in [128, d, 512] n-chunks (one DMA each) so the first
    chunk of K and Q arrives after ~9us of serial DMA instead of ~30us.
  - bk is dropped exactly (softmax shift invariance); bv is folded into
    the host-side output bias (attention weights sum to 1); bq is applied
    on-device during the qh PSUM->SBUF copy.
  - PSUM start_tensor_calc zeroing is bank-granular: only the first
    matmul touching a bank in an accumulation group sets start=True.
"""

import sys
import numpy as np

for _p in ("/opt/trn_rl_repo", "/root/.axon_site/_ro/trn_rl_repo"):
    if _p not in sys.path:
        sys.path.append(_p)

D_MODEL = 1024
NUM_HEADS = 16
DK = 64
B = 2
S = 2048
N_CORES = 8
HPC = 4               # heads per core
E = HPC * DK          # 256 features per core
NQ = 512              # q-chunk size
N_QC = S // NQ        # 4 q chunks
N_KT = S // 128       # 16 k tiles
N_DT = D_MODEL // 128  # 8 contraction tiles for projections

_PROGRAM = None
_RUN_KWARGS = {}      # test harness may set {"trace": True}
_LAST_RESULTS = None  # BassKernelResults of the last run

# Backlog draw schedule: how many deferred projection chunks to emit
# after each (qc, kt) iteration of the attention loop.
_DRAW = {0: [1, 1, 1, 1, 1, 1, 1, 1, 1, 1, 1, 1, 1, 1, 0, 0],
         1: [1, 1, 1, 1] + [0] * 12}


def _build_program():
    import concourse.bass as bass
    import concourse.mybir as mybir
    from concourse import bacc, tile
    from contextlib import ExitStack

    f32 = mybir.dt.float32
    fp16 = mybir.dt.float16
    i16 = mybir.dt.int16
    AF = mybir.ActivationFunctionType
    ALU = mybir.AluOpType
    # Schraudolph fast-exp constants (int16/fp16 bitcast):
    #   i16 = round(s * 0.125 * 1024/ln2 + (15*1024 - C));  C tuned for
    #   min RMS rel error (~1.8%); applied to ~22% of exp tiles on DVE.
    SCH_A = 0.125 * 1024.0 / np.log(2.0)
    SCH_B = 15.0 * 1024.0 - 60.0

    nc = bacc.Bacc("TRN2", target_bir_lowering=False, debug=False,
                   num_devices=N_CORES)

    fp8 = mybir.dt.float8e4
    DR = mybir.MatmulPerfMode.DoubleRow
    xdr = {}
    for nm in ("qTh", "qTl", "kTh", "kTl", "vTh", "vTl"):
        xdr[nm] = nc.dram_tensor(nm, [D_MODEL, S], fp8,
                                 kind="ExternalInput").ap()
    wdr = {}
    for nm in ("wqh", "wql", "wkh", "wkl", "wvh", "wvl"):
        wdr[nm] = nc.dram_tensor(nm, [D_MODEL, E], fp8,
                                 kind="ExternalInput").ap()
    wo = nc.dram_tensor("wo", [E, D_MODEL], fp16, kind="ExternalInput").ap()
    bq = nc.dram_tensor("bq", [E, 1], f32, kind="ExternalInput").ap()
    onesk = nc.dram_tensor("onesk", [128, 1], fp16, kind="ExternalInput").ap()
    zT = nc.dram_tensor("zT", [D_MODEL, S], fp16, kind="ExternalOutput").ap()

    with tile.TileContext(nc) as tc, ExitStack() as ctx:
        persist = ctx.enter_context(tc.tile_pool(name="persist", bufs=1))
        const = ctx.enter_context(tc.tile_pool(name="const", bufs=1))

        w_sb = {}
        for nm in ("wvh", "wvl", "wkh", "wkl", "wqh", "wql"):
            w_sb[nm] = persist.tile([128, N_DT, E], fp8, tag=nm, name=nm)
        wo_sb = persist.tile([128, 2, D_MODEL], fp16, tag="wo", name="wo")
        bq_sb = persist.tile([128, 2], f32, tag="bq", name="bq")

        from concourse.masks import make_identity
        ident = const.tile([128, 128], fp16, tag="ident", name="ident")
        make_identity(nc, ident)
        ones_k = const.tile([128, 1], fp16, tag="ones_k", name="ones_k")

        qh = [persist.tile([128, S], fp16, tag=f"qh{p}", name=f"qh{p}")
              for p in range(2)]
        kh = [persist.tile([128, S], fp16, tag=f"kh{p}", name=f"kh{p}")
              for p in range(2)]
        vh = persist.tile([128, N_KT, E], fp16, tag="vh", name="vh")
        ot = [persist.tile([128, S], fp16, tag=f"ot{p}", name=f"ot{p}")
              for p in range(2)]

        # ---- x chunk tiles + DMA schedule (priority order) --------------
        xpool = ctx.enter_context(tc.tile_pool(name="xpool", bufs=24))
        xt = {}
        for t in ("k", "q", "v"):
            for hl in "hl":
                xt[t + hl] = [xpool.tile([128, N_DT, NQ], fp8, tag="xt",
                                         name=f"x{t}{hl}{n}")
                              for n in range(4)]
        x3 = {k: v.rearrange("(t p) s -> p t s", p=128)
              for k, v in xdr.items()}

        def _xdma(eng, t, hl, n):
            eng.dma_start(xt[t + hl][n][:],
                          x3[t + "T" + hl][:, :, n * NQ:(n + 1) * NQ])

        def _wdma(eng, nm):
            eng.dma_start(w_sb[nm][:],
                          wdr[nm].rearrange("(t p) e -> p t e", p=128))

        # Everything on the SP queue: transfers serialize on the shared DMA
        # device regardless, and any DMA on the scalar queue would block the
        # ACT sequencer from issuing the (critical) exp stream.
        _wdma(nc.sync, "wkh")
        _wdma(nc.sync, "wkl")
        _xdma(nc.sync, "k", "h", 0)
        _xdma(nc.sync, "k", "l", 0)
        _wdma(nc.sync, "wvh")
        _wdma(nc.sync, "wvl")
        nc.sync.dma_start(ones_k[:], onesk)
        _xdma(nc.sync, "v", "h", 0)
        _xdma(nc.sync, "v", "l", 0)
        _wdma(nc.sync, "wqh")
        _wdma(nc.sync, "wql")
        _xdma(nc.sync, "q", "h", 0)
        _xdma(nc.sync, "q", "l", 0)
        nc.sync.dma_start(bq_sb[:], bq.rearrange("(m p) o -> p (m o)", p=128))
        for n in range(1, 4):
            _xdma(nc.sync, "v", "h", n)
            _xdma(nc.sync, "v", "l", n)
            _xdma(nc.sync, "k", "h", n)
            _xdma(nc.sync, "k", "l", n)
            _xdma(nc.sync, "q", "h", n)
            _xdma(nc.sync, "q", "l", n)
        nc.sync.dma_start(wo_sb[:], wo.rearrange("(t p) e -> p t e", p=128))

        # ---- PSUM pools: one shared 8-bank working set ------------------
        scp = ctx.enter_context(tc.tile_pool(name="scp", bufs=4, space="PSUM"))
        scr = ctx.enter_context(tc.tile_pool(name="scr", bufs=1, space="PSUM"))
        outp = ctx.enter_context(tc.tile_pool(name="outp", bufs=1, space="PSUM"))
        sump = ctx.enter_context(tc.tile_pool(name="sump", bufs=1, space="PSUM"))

        ptp = ctx.enter_context(tc.tile_pool(name="ptp", bufs=28))
        rp = ctx.enter_context(tc.tile_pool(name="rp", bufs=4))
        bcp = ctx.enter_context(tc.tile_pool(name="bcp", bufs=12))
        zsbp = ctx.enter_context(tc.tile_pool(name="zsbp", bufs=10))

        # ---- first K/Q n-chunk on the (still idle) score slots ----------
        # weights are host-scaled by 32 (fp8e4 subnormal avoidance); the
        # PSUM->SBUF copy applies the 1/32.  3 passes: wh@xh + wh@xl + wl@xh.
        def _dr_passes(t, wn):
            return ((w_sb[wn + "h"], xt[t + "h"]),
                    (w_sb[wn + "h"], xt[t + "l"]),
                    (w_sb[wn + "l"], xt[t + "h"]))

        def proj_big(t, wn, dst, n, bias):
            for m in range(2):
                ps = scp.tile([128, NQ], f32, tag="sc", name="projbig")
                passes = _dr_passes(t, wn)
                for pi, (wsb, xs) in enumerate(passes):
                    for dp in range(N_DT // 2):
                        nc.tensor.matmul(
                            ps[:], wsb[:, 2 * dp:2 * dp + 2,
                                       m * 128:(m + 1) * 128],
                            xs[n][:, 2 * dp:2 * dp + 2, :],
                            start=(pi == 0 and dp == 0),
                            stop=(pi == 2 and dp == N_DT // 2 - 1),
                            perf_mode=DR)
                if bias is None:
                    nc.vector.tensor_scalar_mul(
                        dst[m][:, n * NQ:(n + 1) * NQ], ps[:], 1.0 / 32)
                else:
                    nc.vector.tensor_scalar(
                        dst[m][:, n * NQ:(n + 1) * NQ], ps[:], 1.0 / 32,
                        bias[:, m:m + 1], ALU.mult, ALU.add)

        # V0-3 run during the kT/qT DMA wait and warm up the PE p-state
        # (they only need wv + the first vT chunk, which load first).
        # Dummy identity transposes (never read) fill the remaining DMA-wait
        # gaps so the p-state ramp reaches full speed before Kn0/Qn0.
        _V_PRE = 4
        wtp = scp.tile([128, 1024], fp16, tag="sc", name="wtp")

        def warm(cnt):
            for i in range(cnt):
                nc.tensor.matmul(
                    wtp[:, (i % 8) * 128:(i % 8 + 1) * 128], ident[:],
                    ident[:], is_transpose=True, start=True, stop=True,
                    skip_group_check=True)

        # ---- deferred projection backlog (drip-fed through scratch) -----
        def emit_v(st0, nst=1):
            vps = scr.tile([128, nst, E], f32, tag="scr", name="vps")
            passes = ((xt["vh"], w_sb["wvh"]), (xt["vl"], w_sb["wvh"]),
                      (xt["vh"], w_sb["wvl"]))
            for stl in range(nst):
                st = st0 + stl
                n, col = st // 4, (st % 4) * 128
                for pi, (xs, wsb) in enumerate(passes):
                    for dp in range(N_DT // 2):
                        nc.tensor.matmul(
                            vps[:, stl, :],
                            xs[n][:, 2 * dp:2 * dp + 2, col:col + 128],
                            wsb[:, 2 * dp:2 * dp + 2, :],
                            start=(pi == 0 and dp == 0 and stl == 0),
                            stop=(pi == 2 and dp == N_DT // 2 - 1),
                            perf_mode=DR, skip_group_check=True)
            nc.vector.tensor_scalar_mul(vh[:, st0:st0 + nst, :], vps[:],
                                        1.0 / 32)

        def emit_kq_chunk(t, wn, dst, n, m, bias, on_sc=False):
            if on_sc:
                ps = scp.tile([128, NQ], f32, tag="sc", name="kqps")
            else:
                ps = scr.tile([128, NQ], f32, tag="scr", name="kqps")
            passes = _dr_passes(t, wn)
            for pi, (wsb, xs) in enumerate(passes):
                for dp in range(N_DT // 2):
                    nc.tensor.matmul(
                        ps[:], wsb[:, 2 * dp:2 * dp + 2,
                                   m * 128:(m + 1) * 128],
                        xs[n][:, 2 * dp:2 * dp + 2, :],
                        start=(pi == 0 and dp == 0),
                        stop=(pi == 2 and dp == N_DT // 2 - 1),
                        perf_mode=DR)
            if bias is None:
                nc.vector.tensor_scalar_mul(
                    dst[m][:, n * NQ:(n + 1) * NQ], ps[:], 1.0 / 32)
            else:
                nc.vector.tensor_scalar(
                    dst[m][:, n * NQ:(n + 1) * NQ], ps[:], 1.0 / 32,
                    bias[:, m:m + 1], ALU.mult, ALU.add)

        warm(20)
        proj_big("k", "wk", kh, 0, None)
        warm(8)
        for st in range(_V_PRE):
            emit_v(st)
        warm(8)
        proj_big("q", "wq", qh, 0, bq_sb)

        backlog = []
        _K = lambda n, m, sc_=False: (
            lambda: emit_kq_chunk("k", "wk", kh, n, m, None, sc_))
        _Q = lambda n, m, sc_=False: (
            lambda: emit_kq_chunk("q", "wq", qh, n, m, bq_sb, sc_))
        _V = lambda st: (lambda: emit_v(st, 2))
        backlog += [_V(4), _K(1, 0, True), _K(1, 1, True), _V(6),
                    _K(2, 0, True), _K(2, 1, True), _V(8), _K(3, 0, True),
                    _V(10), _K(3, 1, True), _V(12), _Q(1, 0, True), _V(14),
                    _Q(1, 1, True), _Q(2, 0, True), _Q(2, 1, True),
                    _Q(3, 0, True), _Q(3, 1, True)]
        backlog = backlog[::-1]  # pop() from the front

        # ---- attention + out_proj, per q-chunk --------------------------
        for qc in range(N_QC):
            q0, q1 = qc * NQ, (qc + 1) * NQ
            out2 = outp.tile([128, 4, E], f32, tag="out2", name="out2")
            sums = sump.tile([128, 16], f32, tag="sums", name="sums")

            def pv_sums(kt, pts):
                # only the FIRST matmul touching each PSUM bank of an
                # accumulation group may set start=True (bank-granular zero)
                for h in range(4):
                    for qt in range(4):
                        lhsT = pts[h][:, qt * 128:(qt + 1) * 128]
                        nc.tensor.matmul(
                            out2[:, qt, h * 64:(h + 1) * 64], lhsT,
                            vh[:, kt, h * 64:(h + 1) * 64],
                            start=(kt == 0 and h == 0 and qt % 2 == 0),
                            stop=(kt == N_KT - 1),
                            skip_group_check=True)
                        nc.tensor.matmul(
                            sums[:, qt * 4 + h:qt * 4 + h + 1], lhsT,
                            ones_k[:],
                            start=(kt == 0 and h == 0 and qt == 0),
                            stop=(kt == N_KT - 1),
                            skip_group_check=True)

            draw = _DRAW.get(qc, [0] * N_KT)
            prev_pts = None
            for kt in range(N_KT):
                k0 = kt * 128
                scs = []
                for h in range(4):
                    p, j = h // 2, h % 2
                    lo, hi = j * 64, (j + 1) * 64
                    sc = scp.tile([128, NQ], f32, tag="sc", name="sc")
                    nc.tensor.matmul(
                        sc[:], kh[p][lo:hi, k0:k0 + 128],
                        qh[p][lo:hi, q0:q1], start=True, stop=True)
                    scs.append(sc)
                if prev_pts is not None:
                    pv_sums(kt - 1, prev_pts)
                for _ in range(draw[kt]):
                    if backlog:
                        backlog.pop()()
                pts = []
                for h in range(4):
                    off = ((kt * 4 + h) % 16 == 9 if qc == 0
                           else (kt * 4 + h) % 8 in (1, 4, 6))
                    if off:
                        pti = ptp.tile([128, NQ], i16, tag="pt", name="pti")
                        nc.vector.tensor_scalar(
                            pti[:], scs[h][:], SCH_A, SCH_B,
                            ALU.mult, ALU.add)
                        pt = pti.bitcast(fp16)
                    else:
                        pt = ptp.tile([128, NQ], fp16, tag="pt", name="pt")
                        nc.scalar.activation(pt[:], scs[h][:], AF.Exp,
                                             scale=0.125)
                    pts.append(pt)
                prev_pts = pts
            pv_sums(N_KT - 1, prev_pts)

            # drain: normalize in [q, e], transpose to [e, q], out_proj.
            # All 16 normalize blocks go first (DVE/ACT alternating on the
            # last chunk), then all transposes, then all ot copies, so the
            # engines pipeline instead of chaining.  The out_proj of qc<3 is
            # deferred into the NEXT q-chunk's backlog (so it never blocks
            # the loop); qc3's runs on the freed score slots.
            last = qc == N_QC - 1
            rv = rp.tile([128, 16], f32, tag="rv", name="rv")
            nc.vector.reciprocal(rv[:], sums[:])
            o2n = []
            for qt in range(4):
                o2 = bcp.tile([128, E], fp16, tag="o2n", name="o2n")
                o2n.append(o2)
            # block-split DVE/ACT on the last chunk (qt0/1 vs qt2/3) so
            # each engine's ops chain densely instead of ping-ponging
            for qt in range(4):
                for h in range(4):
                    c0 = qt * 4 + h
                    if last and qt >= 2:
                        nc.scalar.activation(
                            o2n[qt][:, h * 64:(h + 1) * 64],
                            out2[:, qt, h * 64:(h + 1) * 64],
                            AF.Copy, scale=rv[:, c0:c0 + 1])
                    else:
                        nc.vector.tensor_scalar_mul(
                            o2n[qt][:, h * 64:(h + 1) * 64],
                            out2[:, qt, h * 64:(h + 1) * 64],
                            rv[:, c0:c0 + 1])
            tp = scr.tile([128, 1024], fp16, tag="scr", name="tp")
            for qt in range(4):
                for et in range(2):
                    blk = qt * 2 + et
                    nc.tensor.matmul(
                        tp[:, blk * 128:(blk + 1) * 128],
                        o2n[qt][:, et * 128:(et + 1) * 128], ident[:],
                        is_transpose=True, start=True, stop=True,
                        skip_group_check=True)
            for qt in range(4):
                for et in range(2):
                    blk = qt * 2 + et
                    if last and qt >= 2:
                        nc.scalar.activation(
                            ot[et][:, q0 + qt * 128:q0 + (qt + 1) * 128],
                            tp[:, blk * 128:(blk + 1) * 128],
                            AF.Copy, scale=1.0)
                    else:
                        nc.vector.tensor_copy(
                            ot[et][:, q0 + qt * 128:q0 + (qt + 1) * 128],
                            tp[:, blk * 128:(blk + 1) * 128])

            def out_proj_chunk(qc_, e, on_sc):
                q0_, q1_ = qc_ * NQ, (qc_ + 1) * NQ
                if on_sc:
                    zps = scp.tile([128, NQ], f32, tag="sc", name="zps")
                else:
                    zps = scr.tile([128, NQ], f32, tag="scr", name="zps")
                for c in range(2):
                    nc.tensor.matmul(
                        zps[:], wo_sb[:, c, e * 128:(e + 1) * 128],
                        ot[c][:, q0_:q1_], start=(c == 0), stop=(c == 1))
                zsb = zsbp.tile([128, NQ], fp16, tag="zsb", name="zsb")
                if on_sc and e >= 4:
                    nc.scalar.activation(zsb[:], zps[:], AF.Copy, scale=1.0)
                else:
                    nc.vector.tensor_copy(zsb[:], zps[:])
                nc.sync.dma_start(zT[e * 128:(e + 1) * 128, q0_:q1_], zsb[:])

            if last:
                for e in range(N_DT):
                    out_proj_chunk(qc, e, True)
            else:
                for e in range(N_DT):
                    out_proj_chunk(qc, e, False)

    nc.compile()
    return nc


def _get_program():
    global _PROGRAM
    if _PROGRAM is None:
        _PROGRAM = _build_program()
    return _PROGRAM


ONESK_NP = None


def _init_consts():
    global ONESK_NP
    if ONESK_NP is None:
        ONESK_NP = np.ones((128, 1), np.float16)


def _hilo(a, f8):
    hi = a.astype(f8)
    lo = (a - hi.astype(np.float32)).astype(f8)
    return np.ascontiguousarray(hi), np.ascontiguousarray(lo)


def _make_in_maps(q, k, v, Wq, bq, Wk, Wv, Wo):
    _init_consts()
    import ml_dtypes
    f8 = ml_dtypes.float8_e4m3
    f32 = np.float32
    xT = {}
    for b in range(B):
        for nm, x in (("q", q), ("k", k), ("v", v)):
            h, lo = _hilo(np.ascontiguousarray(x[b].T), f8)
            xT[(nm, b)] = (h, lo)
    wslices = {}
    for g in range(4):
        sl = slice(g * E, (g + 1) * E)
        for nm, W in (("wq", Wq), ("wk", Wk), ("wv", Wv)):
            h, lo = _hilo(np.ascontiguousarray(W[sl, :].T) * 32.0, f8)
            wslices[(nm, g)] = (h, lo)
        wslices[("wo", g)] = np.ascontiguousarray(Wo[:, sl].T, dtype=np.float16)
        wslices[("bq", g)] = np.ascontiguousarray(
            bq[sl].reshape(E, 1), dtype=f32)
    in_maps = []
    for c in range(N_CORES):
        b, g = c // 4, c % 4
        in_maps.append({
            "onesk": ONESK_NP,
            "qTh": xT[("q", b)][0], "qTl": xT[("q", b)][1],
            "kTh": xT[("k", b)][0], "kTl": xT[("k", b)][1],
            "vTh": xT[("v", b)][0], "vTl": xT[("v", b)][1],
            "wqh": wslices[("wq", g)][0], "wql": wslices[("wq", g)][1],
            "wkh": wslices[("wk", g)][0], "wkl": wslices[("wk", g)][1],
            "wvh": wslices[("wv", g)][0], "wvl": wslices[("wv", g)][1],
            "wo": wslices[("wo", g)], "bq": wslices[("bq", g)],
        })
    return in_maps


def _numpy_fallback(q, k, v, mask, Wq, bq, Wk, bk, Wv, bv, Wo, bo):
    # Only used if mask is not all-True (never the case for this problem).
    def proj(x, W, b_):
        y = x @ W.T + b_
        return y.reshape(B, S, NUM_HEADS, DK).transpose(0, 2, 1, 3)
    qh, kh, vh = proj(q, Wq, bq), proj(k, Wk, bk), proj(v, Wv, bv)
    sc = np.einsum("bhqd,bhkd->bhqk", qh, kh) / np.sqrt(DK)
    sc = np.where(mask, sc, np.float32(-1e9))
    sc = sc - sc.max(-1, keepdims=True)
    p = np.exp(sc)
    p /= p.sum(-1, keepdims=True)
    o = np.einsum("bhqk,bhkd->bhqd", p, vh)
    o = o.transpose(0, 2, 1, 3).reshape(B, S, D_MODEL)
    return (o @ Wo.T + bo).astype(np.float32)


def kernel(q, k, v, mask, Wq, bq, Wk, bk, Wv, bv, Wo, bo):
    q = np.asarray(q, dtype=np.float32)
    k = np.asarray(k, dtype=np.float32)
    v = np.asarray(v, dtype=np.float32)
    Wq, Wk, Wv, Wo = (np.asarray(w, dtype=np.float32) for w in (Wq, Wk, Wv, Wo))
    bq, bk, bv, bo = (np.asarray(x, dtype=np.float32) for x in (bq, bk, bv, bo))
    if not np.all(np.asarray(mask)):
        return _numpy_fallback(q, k, v, np.asarray(mask), Wq, bq, Wk, bk,
                               Wv, bv, Wo, bo)

    from concourse.bass_utils import run_bass_kernel_spmd
    nc = _get_program()
    in_maps = _make_in_maps(q, k, v, Wq, bq, Wk, Wv, Wo)
    res = run_bass_kernel_spmd(nc, in_maps, core_ids=list(range(N_CORES)),
                               **_RUN_KWARGS)
    global _LAST_RESULTS
    _LAST_RESULTS = res
    # bk is dropped on-device (exact: softmax shift invariance); bv is
    # folded into the output bias (attention weights sum to 1).
    bo_eff = bo + Wo @ bv
    out = np.empty((B, S, D_MODEL), dtype=np.float32)
    for b in range(B):
        acc = res.results[4 * b]["zT"].astype(np.float32)
        for g in range(1, 4):
            acc = acc + res.results[4 * b + g]["zT"].astype(np.float32)
        out[b] = acc.T + bo_eff
    return out


# revision 41
# speedup vs baseline: 1.0144x; 1.0053x over previous
"""Trainium2 Bass kernel for nn_MultiHeadAttention (B=2, S=2048, d_model=1024, H=16).

Sharding (8 cores): data-parallel over B (2) x tensor-parallel over head groups
(4 groups of 4 heads).  Each core computes its head-group's Q/K/V projections
(column-sharded weights), attention for its 4 heads, and a row-parallel
out_proj partial product.  The host sums the 4 partials per batch (the
"all-reduce") and adds the output bias.

v3 design notes (cost-model driven):
  - P@V uses SWAPPED operands: P (exp scores, [k,q]) stationary, V ([k,e])
    moving, so the moving free dim is 64 instead of 512; attention output
    lands in [q, e].  Softmax denominators ride along as N=1 matmuls
    (rhs = ones) reusing the loaded P stationary tile.
  - Normalization = per-partition scalar multiply on DVE; PE transposes
    bring [q, e] back to [e, q] for the row-parallel out_proj.
  - PSUM is a single 8-bank working set shared by EVERYTHING (no stacked
    stage pools, which would serialize projections before attention):
    sc 2x2 banks, out2 2, sums 1, scratch 1.  Projections beyond the
    first k/q n-chunk are drip-fed through the scratch bank inside the
    attention loop (deadline-ordered backlog), so the ACT exp stream --
    the critical resource -- starts ~15us in instead of ~65us.
  - x is loaded in [128, d, 512] n-chunks (one DMA each) so the first
    chunk of K and Q arrives after ~9us of serial DMA instead of ~30us.
  - bk is dropped exactly (softmax shift invariance); bv is folded into
    the host-side output bias (attention weights sum to 1); bq is applied
    on-device during the qh PSUM->SBUF copy.
  - PSUM start_tensor_calc zeroing is bank-granular: only the first
    matmul touching a bank in an accumulation group sets start=True.
"""

import sys
import numpy as np

for _p in ("/opt/trn_rl_repo", "/root/.axon_site/_ro/trn_rl_repo"):
    if _p not in sys.path:
        sys.path.append(_p)

D_MODEL = 1024
NUM_HEADS = 16
DK = 64
B = 2
S = 2048
N_CORES = 8
HPC = 4               # heads per core
E = HPC * DK          # 256 features per core
NQ = 512              # q-chunk size
N_QC = S // NQ        # 4 q chunks
N_KT = S // 128       # 16 k tiles
N_DT = D_MODEL // 128  # 8 contraction tiles for projections

_PROGRAM = None
_RUN_KWARGS = {}      # test harness may set {"trace": True}
_LAST_RESULTS = None  # BassKernelResults of the last run

# Backlog draw schedule: how many deferred projection chunks to emit
# after each (qc, kt) iteration of the attention loop.
_DRAW = {0: [1, 1, 1, 1, 1, 1, 1, 1, 1, 1, 1, 1, 1, 1, 0, 0],
         1: [1, 1, 1, 1] + [0] * 12}


def _build_program():
    import concourse.bass as bass
    import concourse.mybir as mybir
    from concourse import bacc, tile
    from contextlib import ExitStack

    f32 = mybir.dt.float32
    fp16 = mybir.dt.float16
    i16 = mybir.dt.int16
    AF = mybir.ActivationFunctionType
    ALU = mybir.AluOpType
    # Schraudolph fast-exp constants (int16/fp16 bitcast):
    #   i16 = round(s * 0.125 * 1024/ln2 + (15*1024 - C));  C tuned for
    #   min RMS rel error (~1.8%); applied to ~22% of exp tiles on DVE.
    SCH_A = 0.125 * 1024.0 / np.log(2.0)
    SCH_B = 15.0 * 1024.0 - 60.0

    nc = bacc.Bacc("TRN2", target_bir_lowering=False, debug=False,
                   num_devices=N_CORES)

    fp8 = mybir.dt.float8e4
    DR = mybir.MatmulPerfMode.DoubleRow
    xdr = {}
    for nm in ("qTh", "qTl", "kTh", "kTl", "vTh", "vTl"):
        xdr[nm] = nc.dram_tensor(nm, [D_MODEL, S], fp8,
                                 kind="ExternalInput").ap()
    wdr = {}
    for nm in ("wqh", "wql", "wkh", "wkl", "wvh", "wvl"):
        wdr[nm] = nc.dram_tensor(nm, [D_MODEL, E], fp8,
                                 kind="ExternalInput").ap()
    wo = nc.dram_tensor("wo", [E, D_MODEL], fp16, kind="ExternalInput").ap()
    bq = nc.dram_tensor("bq", [E, 1], f32, kind="ExternalInput").ap()
    onesk = nc.dram_tensor("onesk", [128, 1], fp16, kind="ExternalInput").ap()
    zT = nc.dram_tensor("zT", [D_MODEL, S], fp16, kind="ExternalOutput").ap()

    with tile.TileContext(nc) as tc, ExitStack() as ctx:
        persist = ctx.enter_context(tc.tile_pool(name="persist", bufs=1))
        const = ctx.enter_context(tc.tile_pool(name="const", bufs=1))

        w_sb = {}
        for nm in ("wvh", "wvl", "wkh", "wkl", "wqh", "wql"):
            w_sb[nm] = persist.tile([128, N_DT, E], fp8, tag=nm, name=nm)
        wo_sb = persist.tile([128, 2, D_MODEL], fp16, tag="wo", name="wo")
        bq_sb = persist.tile([128, 2], f32, tag="bq", name="bq")

        from concourse.masks import make_identity
        ident = const.tile([128, 128], fp16, tag="ident", name="ident")
        make_identity(nc, ident)
        ones_k = const.tile([128, 1], fp16, tag="ones_k", name="ones_k")

        qh = [persist.tile([128, S], fp16, tag=f"qh{p}", name=f"qh{p}")
              for p in range(2)]
        kh = [persist.tile([128, S], fp16, tag=f"kh{p}", name=f"kh{p}")
              for p in range(2)]
        vh = persist.tile([128, N_KT, E], fp16, tag="vh", name="vh")
        ot = [persist.tile([128, S], fp16, tag=f"ot{p}", name=f"ot{p}")
              for p in range(2)]

        # ---- x chunk tiles + DMA schedule (priority order) --------------
        xpool = ctx.enter_context(tc.tile_pool(name="xpool", bufs=24))
        xt = {}
        for t in ("k", "q", "v"):
            for hl in "hl":
                xt[t + hl] = [xpool.tile([128, N_DT, NQ], fp8, tag="xt",
                                         name=f"x{t}{hl}{n}")
                              for n in range(4)]
        x3 = {k: v.rearrange("(t p) s -> p t s", p=128)
              for k, v in xdr.items()}

        def _xdma(eng, t, hl, n):
            eng.dma_start(xt[t + hl][n][:],
                          x3[t + "T" + hl][:, :, n * NQ:(n + 1) * NQ])

        def _wdma(eng, nm):
            eng.dma_start(w_sb[nm][:],
                          wdr[nm].rearrange("(t p) e -> p t e", p=128))

        # Everything on the SP queue: transfers serialize on the shared DMA
        # device regardless, and any DMA on the scalar queue would block the
        # ACT sequencer from issuing the (critical) exp stream.
        _wdma(nc.sync, "wkh")
        _wdma(nc.sync, "wkl")
        _wdma(nc.sync, "wvh")
        _wdma(nc.sync, "wvl")
        nc.sync.dma_start(ones_k[:], onesk)
        _xdma(nc.sync, "v", "h", 0)
        _xdma(nc.sync, "v", "l", 0)
        _xdma(nc.sync, "k", "h", 0)
        _xdma(nc.sync, "k", "l", 0)
        _wdma(nc.sync, "wqh")
        _wdma(nc.sync, "wql")
        _xdma(nc.sync, "q", "h", 0)
        _xdma(nc.sync, "q", "l", 0)
        nc.sync.dma_start(bq_sb[:], bq.rearrange("(m p) o -> p (m o)", p=128))
        # K/V chunks feed qc0's in-order backlog; Q chunks are only needed
        # from the second q-chunk on, so they stream last
        for t, n in (("v", 1), ("k", 1), ("v", 2), ("k", 2), ("v", 3),
                     ("k", 3), ("q", 1), ("q", 2), ("q", 3)):
            _xdma(nc.sync, t, "h", n)
            _xdma(nc.sync, t, "l", n)
        nc.sync.dma_start(wo_sb[:], wo.rearrange("(t p) e -> p t e", p=128))

        # ---- PSUM pools: one shared 8-bank working set ------------------
        scp = ctx.enter_context(tc.tile_pool(name="scp", bufs=4, space="PSUM"))
        scr = ctx.enter_context(tc.tile_pool(name="scr", bufs=1, space="PSUM"))
        outp = ctx.enter_context(tc.tile_pool(name="outp", bufs=1, space="PSUM"))
        sump = ctx.enter_context(tc.tile_pool(name="sump", bufs=1, space="PSUM"))

        ptp = ctx.enter_context(tc.tile_pool(name="ptp", bufs=28))
        rp = ctx.enter_context(tc.tile_pool(name="rp", bufs=4))
        bcp = ctx.enter_context(tc.tile_pool(name="bcp", bufs=12))
        zsbp = ctx.enter_context(tc.tile_pool(name="zsbp", bufs=10))

        # ---- first K/Q n-chunk on the (still idle) score slots ----------
        # weights are host-scaled by 32 (fp8e4 subnormal avoidance); the
        # PSUM->SBUF copy applies the 1/32.  3 passes: wh@xh + wh@xl + wl@xh.
        def _dr_passes(t, wn):
            return ((w_sb[wn + "h"], xt[t + "h"]),
                    (w_sb[wn + "h"], xt[t + "l"]),
                    (w_sb[wn + "l"], xt[t + "h"]))

        def proj_big(t, wn, dst, n, bias):
            for m in range(2):
                ps = scp.tile([128, NQ], f32, tag="sc", name="projbig")
                passes = _dr_passes(t, wn)
                for pi, (wsb, xs) in enumerate(passes):
                    for dp in range(N_DT // 2):
                        nc.tensor.matmul(
                            ps[:], wsb[:, 2 * dp:2 * dp + 2,
                                       m * 128:(m + 1) * 128],
                            xs[n][:, 2 * dp:2 * dp + 2, :],
                            start=(pi == 0 and dp == 0),
                            stop=(pi == 2 and dp == N_DT // 2 - 1),
                            perf_mode=DR)
                if bias is None:
                    nc.vector.tensor_scalar_mul(
                        dst[m][:, n * NQ:(n + 1) * NQ], ps[:], 1.0 / 32)
                else:
                    nc.vector.tensor_scalar(
                        dst[m][:, n * NQ:(n + 1) * NQ], ps[:], 1.0 / 32,
                        bias[:, m:m + 1], ALU.mult, ALU.add)

        # V0-3 run during the kT/qT DMA wait and warm up the PE p-state
        # (they only need wv + the first vT chunk, which load first).
        # Dummy identity transposes (never read) fill the remaining DMA-wait
        # gaps so the p-state ramp reaches full speed before Kn0/Qn0.
        _V_PRE = 4
        wtp = scp.tile([128, 1024], fp16, tag="sc", name="wtp")

        def warm(cnt):
            for i in range(cnt):
                nc.tensor.matmul(
                    wtp[:, (i % 8) * 128:(i % 8 + 1) * 128], ident[:],
                    ident[:], is_transpose=True, start=True, stop=True,
                    skip_group_check=True)

        # ---- deferred projection backlog (drip-fed through scratch) -----
        def emit_v(st0, nst=1):
            vps = scr.tile([128, nst, E], f32, tag="scr", name="vps")
            passes = ((xt["vh"], w_sb["wvh"]), (xt["vl"], w_sb["wvh"]),
                      (xt["vh"], w_sb["wvl"]))
            for stl in range(nst):
                st = st0 + stl
                n, col = st // 4, (st % 4) * 128
                for pi, (xs, wsb) in enumerate(passes):
                    for dp in range(N_DT // 2):
                        nc.tensor.matmul(
                            vps[:, stl, :],
                            xs[n][:, 2 * dp:2 * dp + 2, col:col + 128],
                            wsb[:, 2 * dp:2 * dp + 2, :],
                            start=(pi == 0 and dp == 0 and stl == 0),
                            stop=(pi == 2 and dp == N_DT // 2 - 1),
                            perf_mode=DR, skip_group_check=True)
            nc.vector.tensor_scalar_mul(vh[:, st0:st0 + nst, :], vps[:],
                                        1.0 / 32)

        def emit_kq_chunk(t, wn, dst, n, m, bias, on_sc=False):
            if on_sc:
                ps = scp.tile([128, NQ], f32, tag="sc", name="kqps")
            else:
                ps = scr.tile([128, NQ], f32, tag="scr", name="kqps")
            passes = _dr_passes(t, wn)
            for pi, (wsb, xs) in enumerate(passes):
                for dp in range(N_DT // 2):
                    nc.tensor.matmul(
                        ps[:], wsb[:, 2 * dp:2 * dp + 2,
                                   m * 128:(m + 1) * 128],
                        xs[n][:, 2 * dp:2 * dp + 2, :],
                        start=(pi == 0 and dp == 0),
                        stop=(pi == 2 and dp == N_DT // 2 - 1),
                        perf_mode=DR)
            if bias is None:
                nc.vector.tensor_scalar_mul(
                    dst[m][:, n * NQ:(n + 1) * NQ], ps[:], 1.0 / 32)
            else:
                nc.vector.tensor_scalar(
                    dst[m][:, n * NQ:(n + 1) * NQ], ps[:], 1.0 / 32,
                    bias[:, m:m + 1], ALU.mult, ALU.add)

        warm(20)
        proj_big("k", "wk", kh, 0, None)
        warm(8)
        for st in range(_V_PRE):
            emit_v(st)
        warm(8)
        proj_big("q", "wq", qh, 0, bq_sb)

        backlog = []
        _K = lambda n, m, sc_=False: (
            lambda: emit_kq_chunk("k", "wk", kh, n, m, None, sc_))
        _Q = lambda n, m, sc_=False: (
            lambda: emit_kq_chunk("q", "wq", qh, n, m, bq_sb, sc_))
        _V = lambda st: (lambda: emit_v(st, 2))
        backlog += [_V(4), _K(1, 0, True), _K(1, 1, True), _V(6),
                    _K(2, 0, True), _K(2, 1, True), _V(8), _K(3, 0, True),
                    _V(10), _K(3, 1, True), _V(12), _Q(1, 0, True), _V(14),
                    _Q(1, 1, True), _Q(2, 0, True), _Q(2, 1, True),
                    _Q(3, 0, True), _Q(3, 1, True)]
        backlog = backlog[::-1]  # pop() from the front

        # ---- attention + out_proj, per q-chunk --------------------------
        for qc in range(N_QC):
            q0, q1 = qc * NQ, (qc + 1) * NQ
            out2 = outp.tile([128, 4, E], f32, tag="out2", name="out2")
            sums = sump.tile([128, 16], f32, tag="sums", name="sums")

            def pv_sums(kt, pts):
                # only the FIRST matmul touching each PSUM bank of an
                # accumulation group may set start=True (bank-granular zero)
                for h in range(4):
                    for qt in range(4):
                        lhsT = pts[h][:, qt * 128:(qt + 1) * 128]
                        nc.tensor.matmul(
                            out2[:, qt, h * 64:(h + 1) * 64], lhsT,
                            vh[:, kt, h * 64:(h + 1) * 64],
                            start=(kt == 0 and h == 0 and qt % 2 == 0),
                            stop=(kt == N_KT - 1),
                            skip_group_check=True)
                        nc.tensor.matmul(
                            sums[:, qt * 4 + h:qt * 4 + h + 1], lhsT,
                            ones_k[:],
                            start=(kt == 0 and h == 0 and qt == 0),
                            stop=(kt == N_KT - 1),
                            skip_group_check=True)

            draw = _DRAW.get(qc, [0] * N_KT)
            prev_pts = None
            for kt in range(N_KT):
                k0 = kt * 128
                scs = []
                for h in range(4):
                    p, j = h // 2, h % 2
                    lo, hi = j * 64, (j + 1) * 64
                    sc = scp.tile([128, NQ], f32, tag="sc", name="sc")
                    nc.tensor.matmul(
                        sc[:], kh[p][lo:hi, k0:k0 + 128],
                        qh[p][lo:hi, q0:q1], start=True, stop=True)
                    scs.append(sc)
                if prev_pts is not None:
                    pv_sums(kt - 1, prev_pts)
                for _ in range(draw[kt]):
                    if backlog:
                        backlog.pop()()
                pts = []
                for h in range(4):
                    off = ((kt * 4 + h) % 16 == 9 if qc == 0
                           else (kt * 4 + h) % 8 in (1, 4, 6))
                    if off:
                        pti = ptp.tile([128, NQ], i16, tag="pt", name="pti")
                        nc.vector.tensor_scalar(
                            pti[:], scs[h][:], SCH_A, SCH_B,
                            ALU.mult, ALU.add)
                        pt = pti.bitcast(fp16)
                    else:
                        pt = ptp.tile([128, NQ], fp16, tag="pt", name="pt")
                        nc.scalar.activation(pt[:], scs[h][:], AF.Exp,
                                             scale=0.125)
                    pts.append(pt)
                prev_pts = pts
            pv_sums(N_KT - 1, prev_pts)

            # drain: normalize in [q, e], transpose to [e, q], out_proj.
            # All 16 normalize blocks go first (DVE/ACT alternating on the
            # last chunk), then all transposes, then all ot copies, so the
            # engines pipeline instead of chaining.  The out_proj of qc<3 is
            # deferred into the NEXT q-chunk's backlog (so it never blocks
            # the loop); qc3's runs on the freed score slots.
            last = qc == N_QC - 1
            rv = rp.tile([128, 16], f32, tag="rv", name="rv")
            nc.vector.reciprocal(rv[:], sums[:])
            o2n = []
            for qt in range(4):
                o2 = bcp.tile([128, E], fp16, tag="o2n", name="o2n")
                o2n.append(o2)
            # block-split DVE/ACT on the last chunk (qt0/1 vs qt2/3) so
            # each engine's ops chain densely instead of ping-ponging
            for qt in range(4):
                for h in range(4):
                    c0 = qt * 4 + h
                    if last and qt >= 2:
                        nc.scalar.activation(
                            o2n[qt][:, h * 64:(h + 1) * 64],
                            out2[:, qt, h * 64:(h + 1) * 64],
                            AF.Copy, scale=rv[:, c0:c0 + 1])
                    else:
                        nc.vector.tensor_scalar_mul(
                            o2n[qt][:, h * 64:(h + 1) * 64],
                            out2[:, qt, h * 64:(h + 1) * 64],
                            rv[:, c0:c0 + 1])
            tp = scr.tile([128, 1024], fp16, tag="scr", name="tp")
            for qt in range(4):
                for et in range(2):
                    blk = qt * 2 + et
                    nc.tensor.matmul(
                        tp[:, blk * 128:(blk + 1) * 128],
                        o2n[qt][:, et * 128:(et + 1) * 128], ident[:],
                        is_transpose=True, start=True, stop=True,
                        skip_group_check=True)
            for qt in range(4):
                for et in range(2):
                    blk = qt * 2 + et
                    if last and qt >= 2:
                        nc.scalar.activation(
                            ot[et][:, q0 + qt * 128:q0 + (qt + 1) * 128],
                            tp[:, blk * 128:(blk + 1) * 128],
                            AF.Copy, scale=1.0)
                    else:
                        nc.vector.tensor_copy(
                            ot[et][:, q0 + qt * 128:q0 + (qt + 1) * 128],
                            tp[:, blk * 128:(blk + 1) * 128])

            def out_proj_chunk(qc_, e, on_sc):
                q0_, q1_ = qc_ * NQ, (qc_ + 1) * NQ
                if on_sc:
                    zps = scp.tile([128, NQ], f32, tag="sc", name="zps")
                else:
                    zps = scr.tile([128, NQ], f32, tag="scr", name="zps")
                for c in range(2):
                    nc.tensor.matmul(
                        zps[:], wo_sb[:, c, e * 128:(e + 1) * 128],
                        ot[c][:, q0_:q1_], start=(c == 0), stop=(c == 1))
                zsb = zsbp.tile([128, NQ], fp16, tag="zsb", name="zsb")
                if on_sc and e >= 4:
                    nc.scalar.activation(zsb[:], zps[:], AF.Copy, scale=1.0)
                else:
                    nc.vector.tensor_copy(zsb[:], zps[:])
                nc.sync.dma_start(zT[e * 128:(e + 1) * 128, q0_:q1_], zsb[:])

            if last:
                for e in range(N_DT):
                    out_proj_chunk(qc, e, True)
            else:
                for e in range(N_DT):
                    out_proj_chunk(qc, e, False)

    nc.compile()
    return nc


def _get_program():
    global _PROGRAM
    if _PROGRAM is None:
        _PROGRAM = _build_program()
    return _PROGRAM


ONESK_NP = None


def _init_consts():
    global ONESK_NP
    if ONESK_NP is None:
        ONESK_NP = np.ones((128, 1), np.float16)


def _hilo(a, f8):
    hi = a.astype(f8)
    lo = (a - hi.astype(np.float32)).astype(f8)
    return np.ascontiguousarray(hi), np.ascontiguousarray(lo)


def _make_in_maps(q, k, v, Wq, bq, Wk, Wv, Wo):
    _init_consts()
    import ml_dtypes
    f8 = ml_dtypes.float8_e4m3
    f32 = np.float32
    xT = {}
    for b in range(B):
        for nm, x in (("q", q), ("k", k), ("v", v)):
            h, lo = _hilo(np.ascontiguousarray(x[b].T), f8)
            xT[(nm, b)] = (h, lo)
    wslices = {}
    for g in range(4):
        sl = slice(g * E, (g + 1) * E)
        for nm, W in (("wq", Wq), ("wk", Wk), ("wv", Wv)):
            h, lo = _hilo(np.ascontiguousarray(W[sl, :].T) * 32.0, f8)
            wslices[(nm, g)] = (h, lo)
        wslices[("wo", g)] = np.ascontiguousarray(Wo[:, sl].T, dtype=np.float16)
        wslices[("bq", g)] = np.ascontiguousarray(
            bq[sl].reshape(E, 1), dtype=f32)
    in_maps = []
    for c in range(N_CORES):
        b, g = c // 4, c % 4
        in_maps.append({
            "onesk": ONESK_NP,
            "qTh": xT[("q", b)][0], "qTl": xT[("q", b)][1],
            "kTh": xT[("k", b)][0], "kTl": xT[("k", b)][1],
            "vTh": xT[("v", b)][0], "vTl": xT[("v", b)][1],
            "wqh": wslices[("wq", g)][0], "wql": wslices[("wq", g)][1],
            "wkh": wslices[("wk", g)][0], "wkl": wslices[("wk", g)][1],
            "wvh": wslices[("wv", g)][0], "wvl": wslices[("wv", g)][1],
            "wo": wslices[("wo", g)], "bq": wslices[("bq", g)],
        })
    return in_maps


def _numpy_fallback(q, k, v, mask, Wq, bq, Wk, bk, Wv, bv, Wo, bo):
    # Only used if mask is not all-True (never the case for this problem).
    def proj(x, W, b_):
        y = x @ W.T + b_
        return y.reshape(B, S, NUM_HEADS, DK).transpose(0, 2, 1, 3)
    qh, kh, vh = proj(q, Wq, bq), proj(k, Wk, bk), proj(v, Wv, bv)
    sc = np.einsum("bhqd,bhkd->bhqk", qh, kh) / np.sqrt(DK)
    sc = np.where(mask, sc, np.float32(-1e9))
    sc = sc - sc.max(-1, keepdims=True)
    p = np.exp(sc)
    p /= p.sum(-1, keepdims=True)
    o = np.einsum("bhqk,bhkd->bhqd", p, vh)
    o = o.transpose(0, 2, 1, 3).reshape(B, S, D_MODEL)
    return (o @ Wo.T + bo).astype(np.float32)


def kernel(q, k, v, mask, Wq, bq, Wk, bk, Wv, bv, Wo, bo):
    q = np.asarray(q, dtype=np.float32)
    k = np.asarray(k, dtype=np.float32)
    v = np.asarray(v, dtype=np.float32)
    Wq, Wk, Wv, Wo = (np.asarray(w, dtype=np.float32) for w in (Wq, Wk, Wv, Wo))
    bq, bk, bv, bo = (np.asarray(x, dtype=np.float32) for x in (bq, bk, bv, bo))
    if not np.all(np.asarray(mask)):
        return _numpy_fallback(q, k, v, np.asarray(mask), Wq, bq, Wk, bk,
                               Wv, bv, Wo, bo)

    from concourse.bass_utils import run_bass_kernel_spmd
    nc = _get_program()
    in_maps = _make_in_maps(q, k, v, Wq, bq, Wk, Wv, Wo)
    res = run_bass_kernel_spmd(nc, in_maps, core_ids=list(range(N_CORES)),
                               **_RUN_KWARGS)
    global _LAST_RESULTS
    _LAST_RESULTS = res
    # bk is dropped on-device (exact: softmax shift invariance); bv is
    # folded into the output bias (attention weights sum to 1).
    bo_eff = bo + Wo @ bv
    out = np.empty((B, S, D_MODEL), dtype=np.float32)
    for b in range(B):
        acc = res.results[4 * b]["zT"].astype(np.float32)
        for g in range(1, 4):
            acc = acc + res.results[4 * b + g]["zT"].astype(np.float32)
        out[b] = acc.T + bo_eff
    return out


# revision 45
# speedup vs baseline: 1.0206x; 1.0061x over previous
"""Trainium2 Bass kernel for nn_MultiHeadAttention (B=2, S=2048, d_model=1024, H=16).

Sharding (8 cores): data-parallel over B (2) x tensor-parallel over head groups
(4 groups of 4 heads).  Each core computes its head-group's Q/K/V projections
(column-sharded weights), attention for its 4 heads, and a row-parallel
out_proj partial product.  The host sums the 4 partials per batch (the
"all-reduce") and adds the output bias.

v3 design notes (cost-model driven):
  - P@V uses SWAPPED operands: P (exp scores, [k,q]) stationary, V ([k,e])
    moving, so the moving free dim is 64 instead of 512; attention output
    lands in [q, e].  Softmax denominators ride along as N=1 matmuls
    (rhs = ones) reusing the loaded P stationary tile.
  - Normalization = per-partition scalar multiply on DVE; PE transposes
    bring [q, e] back to [e, q] for the row-parallel out_proj.
  - PSUM is a single 8-bank working set shared by EVERYTHING (no stacked
    stage pools, which would serialize projections before attention):
    sc 2x2 banks, out2 2, sums 1, scratch 1.  Projections beyond the
    first k/q n-chunk are drip-fed through the scratch bank inside the
    attention loop (deadline-ordered backlog), so the ACT exp stream --
    the critical resource -- starts ~15us in instead of ~65us.
  - x is loaded in [128, d, 512] n-chunks (one DMA each) so the first
    chunk of K and Q arrives after ~9us of serial DMA instead of ~30us.
  - bk is dropped exactly (softmax shift invariance); bv is folded into
    the host-side output bias (attention weights sum to 1); bq is applied
    on-device during the qh PSUM->SBUF copy.
  - PSUM start_tensor_calc zeroing is bank-granular: only the first
    matmul touching a bank in an accumulation group sets start=True.
"""

import sys
import numpy as np

for _p in ("/opt/trn_rl_repo", "/root/.axon_site/_ro/trn_rl_repo"):
    if _p not in sys.path:
        sys.path.append(_p)

D_MODEL = 1024
NUM_HEADS = 16
DK = 64
B = 2
S = 2048
N_CORES = 8
HPC = 4               # heads per core
E = HPC * DK          # 256 features per core
NQ = 512              # q-chunk size
N_QC = S // NQ        # 4 q chunks
N_KT = S // 128       # 16 k tiles
N_DT = D_MODEL // 128  # 8 contraction tiles for projections

_PROGRAM = None
_RUN_KWARGS = {}      # test harness may set {"trace": True}
_LAST_RESULTS = None  # BassKernelResults of the last run

# Backlog draw schedule: how many deferred projection chunks to emit
# after each (qc, kt) iteration of the attention loop.
_DRAW = {0: [1, 1, 1, 1, 1, 1, 1, 1, 1, 1, 1, 1, 1, 1, 0, 0],
         1: [1, 1, 1, 1] + [0] * 12}


def _build_program():
    import concourse.bass as bass
    import concourse.mybir as mybir
    from concourse import bacc, tile
    from contextlib import ExitStack

    f32 = mybir.dt.float32
    fp16 = mybir.dt.float16
    i16 = mybir.dt.int16
    AF = mybir.ActivationFunctionType
    ALU = mybir.AluOpType
    # Schraudolph fast-exp constants (int16/fp16 bitcast):
    #   i16 = round(s * 0.125 * 1024/ln2 + (15*1024 - C));  C tuned for
    #   min RMS rel error (~1.8%); applied to ~22% of exp tiles on DVE.
    SCH_A = 0.125 * 1024.0 / np.log(2.0)
    SCH_B = 15.0 * 1024.0 - 60.0

    nc = bacc.Bacc("TRN2", target_bir_lowering=False, debug=False,
                   num_devices=N_CORES)

    fp8 = mybir.dt.float8e4
    DR = mybir.MatmulPerfMode.DoubleRow
    xdr = {}
    for nm in ("qTh", "qTl", "kTh", "kTl", "vTh", "vTl"):
        xdr[nm] = nc.dram_tensor(nm, [D_MODEL, S], fp8,
                                 kind="ExternalInput").ap()
    wdr = {}
    for nm in ("wqh", "wql", "wkh", "wkl", "wvh", "wvl"):
        wdr[nm] = nc.dram_tensor(nm, [D_MODEL, E], fp8,
                                 kind="ExternalInput").ap()
    wo = nc.dram_tensor("wo", [E, D_MODEL], fp16, kind="ExternalInput").ap()
    bq = nc.dram_tensor("bq", [E, 1], f32, kind="ExternalInput").ap()
    onesk = nc.dram_tensor("onesk", [128, 1], fp16, kind="ExternalInput").ap()
    zT = nc.dram_tensor("zT", [D_MODEL, S], fp16, kind="ExternalOutput").ap()

    with tile.TileContext(nc) as tc, ExitStack() as ctx:
        persist = ctx.enter_context(tc.tile_pool(name="persist", bufs=1))
        const = ctx.enter_context(tc.tile_pool(name="const", bufs=1))

        w_sb = {}
        for nm in ("wvh", "wvl", "wkh", "wkl", "wqh", "wql"):
            w_sb[nm] = persist.tile([128, N_DT, E], fp8, tag=nm, name=nm)
        wo_sb = persist.tile([128, 2, D_MODEL], fp16, tag="wo", name="wo")
        bq_sb = persist.tile([128, 2], f32, tag="bq", name="bq")

        from concourse.masks import make_identity
        ident = const.tile([128, 128], fp16, tag="ident", name="ident")
        make_identity(nc, ident)
        ones_k = const.tile([128, 1], fp16, tag="ones_k", name="ones_k")

        qh = [persist.tile([128, S], fp16, tag=f"qh{p}", name=f"qh{p}")
              for p in range(2)]
        kh = [persist.tile([128, S], fp16, tag=f"kh{p}", name=f"kh{p}")
              for p in range(2)]
        vh = persist.tile([128, N_KT, E], fp16, tag="vh", name="vh")
        ot = [persist.tile([128, S], fp16, tag=f"ot{p}", name=f"ot{p}")
              for p in range(2)]

        # ---- x chunk tiles + DMA schedule (priority order) --------------
        xpool = ctx.enter_context(tc.tile_pool(name="xpool", bufs=24))
        xt = {}
        for t in ("k", "q", "v"):
            for hl in "hl":
                xt[t + hl] = [xpool.tile([128, N_DT, NQ], fp8, tag="xt",
                                         name=f"x{t}{hl}{n}")
                              for n in range(4)]
        x3 = {k: v.rearrange("(t p) s -> p t s", p=128)
              for k, v in xdr.items()}

        def _xdma(eng, t, hl, n):
            eng.dma_start(xt[t + hl][n][:],
                          x3[t + "T" + hl][:, :, n * NQ:(n + 1) * NQ])

        def _wdma(eng, nm):
            eng.dma_start(w_sb[nm][:],
                          wdr[nm].rearrange("(t p) e -> p t e", p=128))

        # Everything on the SP queue: transfers serialize on the shared DMA
        # device regardless, and any DMA on the scalar queue would block the
        # ACT sequencer from issuing the (critical) exp stream.
        _wdma(nc.sync, "wkh")
        _wdma(nc.sync, "wkl")
        _wdma(nc.sync, "wvh")
        _wdma(nc.sync, "wvl")
        nc.sync.dma_start(ones_k[:], onesk)
        _xdma(nc.sync, "v", "h", 0)
        _xdma(nc.sync, "v", "l", 0)
        _xdma(nc.sync, "k", "h", 0)
        _xdma(nc.sync, "k", "l", 0)
        _wdma(nc.sync, "wqh")
        _wdma(nc.sync, "wql")
        _xdma(nc.sync, "q", "h", 0)
        _xdma(nc.sync, "q", "l", 0)
        nc.sync.dma_start(bq_sb[:], bq.rearrange("(m p) o -> p (m o)", p=128))
        # K/V chunks feed qc0's in-order backlog; Q chunks are only needed
        # from the second q-chunk on, so they stream last
        for t, n in (("v", 1), ("k", 1), ("v", 2), ("k", 2), ("v", 3),
                     ("k", 3), ("q", 1), ("q", 2), ("q", 3)):
            _xdma(nc.sync, t, "h", n)
            _xdma(nc.sync, t, "l", n)
        nc.sync.dma_start(wo_sb[:], wo.rearrange("(t p) e -> p t e", p=128))

        # ---- PSUM pools: one shared 8-bank working set ------------------
        scp = ctx.enter_context(tc.tile_pool(name="scp", bufs=4, space="PSUM"))
        scr = ctx.enter_context(tc.tile_pool(name="scr", bufs=1, space="PSUM"))
        outp = ctx.enter_context(tc.tile_pool(name="outp", bufs=1, space="PSUM"))
        sump = ctx.enter_context(tc.tile_pool(name="sump", bufs=1, space="PSUM"))

        ptp = ctx.enter_context(tc.tile_pool(name="ptp", bufs=28))
        rp = ctx.enter_context(tc.tile_pool(name="rp", bufs=4))
        bcp = ctx.enter_context(tc.tile_pool(name="bcp", bufs=12))
        zsbp = ctx.enter_context(tc.tile_pool(name="zsbp", bufs=10))

        # ---- first K/Q n-chunk on the (still idle) score slots ----------
        # weights are host-scaled by 32 (fp8e4 subnormal avoidance); the
        # PSUM->SBUF copy applies the 1/32.  3 passes: wh@xh + wh@xl + wl@xh.
        def _dr_passes(t, wn):
            return ((w_sb[wn + "h"], xt[t + "h"]),
                    (w_sb[wn + "h"], xt[t + "l"]),
                    (w_sb[wn + "l"], xt[t + "h"]))

        def proj_big(t, wn, dst, n, bias):
            for m in range(2):
                ps = scp.tile([128, NQ], f32, tag="sc", name="projbig")
                passes = _dr_passes(t, wn)
                for pi, (wsb, xs) in enumerate(passes):
                    for dp in range(N_DT // 2):
                        nc.tensor.matmul(
                            ps[:], wsb[:, 2 * dp:2 * dp + 2,
                                       m * 128:(m + 1) * 128],
                            xs[n][:, 2 * dp:2 * dp + 2, :],
                            start=(pi == 0 and dp == 0),
                            stop=(pi == 2 and dp == N_DT // 2 - 1),
                            perf_mode=DR)
                if bias is None:
                    nc.vector.tensor_scalar_mul(
                        dst[m][:, n * NQ:(n + 1) * NQ], ps[:], 1.0 / 32)
                else:
                    nc.vector.tensor_scalar(
                        dst[m][:, n * NQ:(n + 1) * NQ], ps[:], 1.0 / 32,
                        bias[:, m:m + 1], ALU.mult, ALU.add)

        # V0-3 run during the kT/qT DMA wait and warm up the PE p-state
        # (they only need wv + the first vT chunk, which load first).
        # Dummy identity transposes (never read) fill the remaining DMA-wait
        # gaps so the p-state ramp reaches full speed before Kn0/Qn0.
        _V_PRE = 4
        wtp = scp.tile([128, 1024], fp16, tag="sc", name="wtp")

        def warm(cnt):
            for i in range(cnt):
                nc.tensor.matmul(
                    wtp[:, (i % 8) * 128:(i % 8 + 1) * 128], ident[:],
                    ident[:], is_transpose=True, start=True, stop=True,
                    skip_group_check=True)

        # ---- deferred projection backlog (drip-fed through scratch) -----
        def emit_v(st0, nst=1):
            vps = scr.tile([128, nst, E], f32, tag="scr", name="vps")
            passes = ((xt["vh"], w_sb["wvh"]), (xt["vl"], w_sb["wvh"]),
                      (xt["vh"], w_sb["wvl"]))
            for stl in range(nst):
                st = st0 + stl
                n, col = st // 4, (st % 4) * 128
                for pi, (xs, wsb) in enumerate(passes):
                    for dp in range(N_DT // 2):
                        nc.tensor.matmul(
                            vps[:, stl, :],
                            xs[n][:, 2 * dp:2 * dp + 2, col:col + 128],
                            wsb[:, 2 * dp:2 * dp + 2, :],
                            start=(pi == 0 and dp == 0 and stl == 0),
                            stop=(pi == 2 and dp == N_DT // 2 - 1),
                            perf_mode=DR, skip_group_check=True)
            nc.vector.tensor_scalar_mul(vh[:, st0:st0 + nst, :], vps[:],
                                        1.0 / 32)

        def emit_kq_chunk(t, wn, dst, n, m, bias, on_sc=False):
            if on_sc:
                ps = scp.tile([128, NQ], f32, tag="sc", name="kqps")
            else:
                ps = scr.tile([128, NQ], f32, tag="scr", name="kqps")
            passes = _dr_passes(t, wn)
            for pi, (wsb, xs) in enumerate(passes):
                for dp in range(N_DT // 2):
                    nc.tensor.matmul(
                        ps[:], wsb[:, 2 * dp:2 * dp + 2,
                                   m * 128:(m + 1) * 128],
                        xs[n][:, 2 * dp:2 * dp + 2, :],
                        start=(pi == 0 and dp == 0),
                        stop=(pi == 2 and dp == N_DT // 2 - 1),
                        perf_mode=DR)
            if bias is None:
                nc.vector.tensor_scalar_mul(
                    dst[m][:, n * NQ:(n + 1) * NQ], ps[:], 1.0 / 32)
            else:
                nc.vector.tensor_scalar(
                    dst[m][:, n * NQ:(n + 1) * NQ], ps[:], 1.0 / 32,
                    bias[:, m:m + 1], ALU.mult, ALU.add)

        warm(20)
        proj_big("k", "wk", kh, 0, None)
        warm(8)
        for st in range(_V_PRE):
            emit_v(st)
        warm(8)
        proj_big("q", "wq", qh, 0, bq_sb)

        backlog = []
        _K = lambda n, m, sc_=False: (
            lambda: emit_kq_chunk("k", "wk", kh, n, m, None, sc_))
        _Q = lambda n, m, sc_=False: (
            lambda: emit_kq_chunk("q", "wq", qh, n, m, bq_sb, sc_))
        _V = lambda st: (lambda: emit_v(st, 2))
        backlog += [_V(4), _K(1, 0, True), _K(1, 1, True), _V(6),
                    _K(2, 0, True), _K(2, 1, True), _V(8), _K(3, 0, True),
                    _V(10), _K(3, 1, True), _V(12), _Q(1, 0, True), _V(14),
                    _Q(1, 1, True), _Q(2, 0, True), _Q(2, 1, True),
                    _Q(3, 0, True), _Q(3, 1, True)]
        backlog = backlog[::-1]  # pop() from the front

        # ---- attention + out_proj, per q-chunk --------------------------
        for qc in range(N_QC):
            q0, q1 = qc * NQ, (qc + 1) * NQ
            out2 = outp.tile([128, 4, E], f32, tag="out2", name="out2")
            sums = sump.tile([128, 16], f32, tag="sums", name="sums")

            def pv_sums(kt, pts):
                # only the FIRST matmul touching each PSUM bank of an
                # accumulation group may set start=True (bank-granular zero)
                for h in range(4):
                    for qt in range(4):
                        lhsT = pts[h][:, qt * 128:(qt + 1) * 128]
                        nc.tensor.matmul(
                            out2[:, qt, h * 64:(h + 1) * 64], lhsT,
                            vh[:, kt, h * 64:(h + 1) * 64],
                            start=(kt == 0 and h == 0 and qt % 2 == 0),
                            stop=(kt == N_KT - 1),
                            skip_group_check=True)
                        nc.tensor.matmul(
                            sums[:, qt * 4 + h:qt * 4 + h + 1], lhsT,
                            ones_k[:],
                            start=(kt == 0 and h == 0 and qt == 0),
                            stop=(kt == N_KT - 1),
                            skip_group_check=True)

            draw = _DRAW.get(qc, [0] * N_KT)
            prev_pts = None
            for kt in range(N_KT):
                k0 = kt * 128
                scs = []
                for h in range(4):
                    p, j = h // 2, h % 2
                    lo, hi = j * 64, (j + 1) * 64
                    sc = scp.tile([128, NQ], f32, tag="sc", name="sc")
                    nc.tensor.matmul(
                        sc[:], kh[p][lo:hi, k0:k0 + 128],
                        qh[p][lo:hi, q0:q1], start=True, stop=True)
                    scs.append(sc)
                if prev_pts is not None:
                    pv_sums(kt - 1, prev_pts)
                for _ in range(draw[kt]):
                    if backlog:
                        backlog.pop()()
                pts = []
                for h in range(4):
                    off = (((kt * 4 + h) % 16 == 9 if kt < 7 else
                            (kt * 4 + h) % 8 in (1, 4, 6)) if qc == 0
                           else (kt * 4 + h) % 8 in (1, 4, 6))
                    if off:
                        pti = ptp.tile([128, NQ], i16, tag="pt", name="pti")
                        nc.vector.tensor_scalar(
                            pti[:], scs[h][:], SCH_A, SCH_B,
                            ALU.mult, ALU.add)
                        pt = pti.bitcast(fp16)
                    else:
                        pt = ptp.tile([128, NQ], fp16, tag="pt", name="pt")
                        nc.scalar.activation(pt[:], scs[h][:], AF.Exp,
                                             scale=0.125)
                    pts.append(pt)
                prev_pts = pts
            pv_sums(N_KT - 1, prev_pts)

            # drain: normalize in [q, e], transpose to [e, q], out_proj.
            # All 16 normalize blocks go first (DVE/ACT alternating on the
            # last chunk), then all transposes, then all ot copies, so the
            # engines pipeline instead of chaining.  The out_proj of qc<3 is
            # deferred into the NEXT q-chunk's backlog (so it never blocks
            # the loop); qc3's runs on the freed score slots.
            last = qc == N_QC - 1
            rv = rp.tile([128, 16], f32, tag="rv", name="rv")
            nc.vector.reciprocal(rv[:], sums[:])
            o2n = []
            for qt in range(4):
                o2 = bcp.tile([128, E], fp16, tag="o2n", name="o2n")
                o2n.append(o2)
            # block-split DVE/ACT on the last chunk (qt0/1 vs qt2/3) so
            # each engine's ops chain densely instead of ping-ponging
            for qt in range(4):
                for h in range(4):
                    c0 = qt * 4 + h
                    if last and qt >= 2:
                        nc.scalar.activation(
                            o2n[qt][:, h * 64:(h + 1) * 64],
                            out2[:, qt, h * 64:(h + 1) * 64],
                            AF.Copy, scale=rv[:, c0:c0 + 1])
                    else:
                        nc.vector.tensor_scalar_mul(
                            o2n[qt][:, h * 64:(h + 1) * 64],
                            out2[:, qt, h * 64:(h + 1) * 64],
                            rv[:, c0:c0 + 1])
            tp = scr.tile([128, 1024], fp16, tag="scr", name="tp")
            for qt in range(4):
                for et in range(2):
                    blk = qt * 2 + et
                    nc.tensor.matmul(
                        tp[:, blk * 128:(blk + 1) * 128],
                        o2n[qt][:, et * 128:(et + 1) * 128], ident[:],
                        is_transpose=True, start=True, stop=True,
                        skip_group_check=True)
            for qt in range(4):
                for et in range(2):
                    blk = qt * 2 + et
                    if last and qt >= 2:
                        nc.scalar.activation(
                            ot[et][:, q0 + qt * 128:q0 + (qt + 1) * 128],
                            tp[:, blk * 128:(blk + 1) * 128],
                            AF.Copy, scale=1.0)
                    else:
                        nc.vector.tensor_copy(
                            ot[et][:, q0 + qt * 128:q0 + (qt + 1) * 128],
                            tp[:, blk * 128:(blk + 1) * 128])

            def out_proj_chunk(qc_, e, on_sc):
                q0_, q1_ = qc_ * NQ, (qc_ + 1) * NQ
                if on_sc:
                    zps = scp.tile([128, NQ], f32, tag="sc", name="zps")
                else:
                    zps = scr.tile([128, NQ], f32, tag="scr", name="zps")
                for c in range(2):
                    nc.tensor.matmul(
                        zps[:], wo_sb[:, c, e * 128:(e + 1) * 128],
                        ot[c][:, q0_:q1_], start=(c == 0), stop=(c == 1))
                zsb = zsbp.tile([128, NQ], fp16, tag="zsb", name="zsb")
                if on_sc and e >= 4:
                    nc.scalar.activation(zsb[:], zps[:], AF.Copy, scale=1.0)
                else:
                    nc.vector.tensor_copy(zsb[:], zps[:])
                nc.sync.dma_start(zT[e * 128:(e + 1) * 128, q0_:q1_], zsb[:])

            if last:
                for e in range(N_DT):
                    out_proj_chunk(qc, e, True)
            else:
                for e in range(N_DT):
                    out_proj_chunk(qc, e, False)

    nc.compile()
    return nc


def _get_program():
    global _PROGRAM
    if _PROGRAM is None:
        _PROGRAM = _build_program()
    return _PROGRAM


ONESK_NP = None


def _init_consts():
    global ONESK_NP
    if ONESK_NP is None:
        ONESK_NP = np.ones((128, 1), np.float16)


def _hilo(a, f8):
    hi = a.astype(f8)
    lo = (a - hi.astype(np.float32)).astype(f8)
    return np.ascontiguousarray(hi), np.ascontiguousarray(lo)


def _make_in_maps(q, k, v, Wq, bq, Wk, Wv, Wo):
    _init_consts()
    import ml_dtypes
    f8 = ml_dtypes.float8_e4m3
    f32 = np.float32
    xT = {}
    for b in range(B):
        for nm, x in (("q", q), ("k", k), ("v", v)):
            h, lo = _hilo(np.ascontiguousarray(x[b].T), f8)
            xT[(nm, b)] = (h, lo)
    wslices = {}
    for g in range(4):
        sl = slice(g * E, (g + 1) * E)
        for nm, W in (("wq", Wq), ("wk", Wk), ("wv", Wv)):
            h, lo = _hilo(np.ascontiguousarray(W[sl, :].T) * 32.0, f8)
            wslices[(nm, g)] = (h, lo)
        wslices[("wo", g)] = np.ascontiguousarray(Wo[:, sl].T, dtype=np.float16)
        wslices[("bq", g)] = np.ascontiguousarray(
            bq[sl].reshape(E, 1), dtype=f32)
    in_maps = []
    for c in range(N_CORES):
        b, g = c // 4, c % 4
        in_maps.append({
            "onesk": ONESK_NP,
            "qTh": xT[("q", b)][0], "qTl": xT[("q", b)][1],
            "kTh": xT[("k", b)][0], "kTl": xT[("k", b)][1],
            "vTh": xT[("v", b)][0], "vTl": xT[("v", b)][1],
            "wqh": wslices[("wq", g)][0], "wql": wslices[("wq", g)][1],
            "wkh": wslices[("wk", g)][0], "wkl": wslices[("wk", g)][1],
            "wvh": wslices[("wv", g)][0], "wvl": wslices[("wv", g)][1],
            "wo": wslices[("wo", g)], "bq": wslices[("bq", g)],
        })
    return in_maps


def _numpy_fallback(q, k, v, mask, Wq, bq, Wk, bk, Wv, bv, Wo, bo):
    # Only used if mask is not all-True (never the case for this problem).
    def proj(x, W, b_):
        y = x @ W.T + b_
        return y.reshape(B, S, NUM_HEADS, DK).transpose(0, 2, 1, 3)
    qh, kh, vh = proj(q, Wq, bq), proj(k, Wk, bk), proj(v, Wv, bv)
    sc = np.einsum("bhqd,bhkd->bhqk", qh, kh) / np.sqrt(DK)
    sc = np.where(mask, sc, np.float32(-1e9))
    sc = sc - sc.max(-1, keepdims=True)
    p = np.exp(sc)
    p /= p.sum(-1, keepdims=True)
    o = np.einsum("bhqk,bhkd->bhqd", p, vh)
    o = o.transpose(0, 2, 1, 3).reshape(B, S, D_MODEL)
    return (o @ Wo.T + bo).astype(np.float32)


def kernel(q, k, v, mask, Wq, bq, Wk, bk, Wv, bv, Wo, bo):
    q = np.asarray(q, dtype=np.float32)
    k = np.asarray(k, dtype=np.float32)
    v = np.asarray(v, dtype=np.float32)
    Wq, Wk, Wv, Wo = (np.asarray(w, dtype=np.float32) for w in (Wq, Wk, Wv, Wo))
    bq, bk, bv, bo = (np.asarray(x, dtype=np.float32) for x in (bq, bk, bv, bo))
    if not np.all(np.asarray(mask)):
        return _numpy_fallback(q, k, v, np.asarray(mask), Wq, bq, Wk, bk,
                               Wv, bv, Wo, bo)

    from concourse.bass_utils import run_bass_kernel_spmd
    nc = _get_program()
    in_maps = _make_in_maps(q, k, v, Wq, bq, Wk, Wv, Wo)
    res = run_bass_kernel_spmd(nc, in_maps, core_ids=list(range(N_CORES)),
                               **_RUN_KWARGS)
    global _LAST_RESULTS
    _LAST_RESULTS = res
    # bk is dropped on-device (exact: softmax shift invariance); bv is
    # folded into the output bias (attention weights sum to 1).
    bo_eff = bo + Wo @ bv
    out = np.empty((B, S, D_MODEL), dtype=np.float32)
    for b in range(B):
        acc = res.results[4 * b]["zT"].astype(np.float32)
        for g in range(1, 4):
            acc = acc + res.results[4 * b + g]["zT"].astype(np.float32)
        out[b] = acc.T + bo_eff
    return out
